# revision 1
# baseline (speedup 1.0000x reference)
"""Multi-head causal attention on 8 TRN2 NeuronCores.

Problem: x[4,2048,1024] @ Wqkv.T -> 16-head causal attention -> @ Wout.T.

Sharding: core c handles batch b=c//2, head-group g=c%2 (8 heads of 64).
Each core computes qkv for its (batch, head-group) slice, causal attention,
and a partial out-projection over its 512 columns of Wout's input dim.
Host sums the two partials per batch (the all-reduce of the hint).

Per-core layouts (host pre-transposes so every matmul contraction dim lands
on SBUF partitions):
  xT   [1024 d, 2048 t]      wqkT [1024 d, 1024 (q|k)e]
  wvT  [1024 d,  512 e]      woT  [ 512 e, 1024 f]
All matmuls run fp32r (1 cycle/row at N>=256 vs 4 for fp32; ~1e-4 rel err).

Emission is software-pipelined to keep the PE dense (HAM stays at 2.4GHz):
the QKV-production matmul groups for t-chunk tc+1 are interleaved into the
attention phase of chunk tc as PE filler between head pairs; S^T for jb+1
is emitted before AV of jb so the PE never waits on the ACT exp.
"""

import sys

sys.path.insert(0, "/opt/trn_rl_repo")

import numpy as np

B, T, D, H = 4, 2048, 1024, 16
E = 512  # per-core head width (8 heads x 64)
ND = 8  # d chunks of 128
NTC = 4  # t chunks of 512
SCALE = 0.125  # 1/sqrt(64)

_NC_CACHE = {}


def build():
    if "nc" in _NC_CACHE:
        return _NC_CACHE["nc"]
    import concourse.bacc as bacc
    import concourse.mybir as mybir
    import concourse.tile as tile

    F32 = mybir.dt.float32
    F32R = mybir.dt.float32r
    EXP = mybir.ActivationFunctionType.Exp

    nc = bacc.Bacc("TRN2", target_bir_lowering=False, debug=False, num_devices=8)
    xT = nc.declare_dram_parameter("xT", [D, T], F32R, isOutput=False)
    wqkT = nc.declare_dram_parameter("wqkT", [D, 2 * E], F32R, isOutput=False)
    wvT = nc.declare_dram_parameter("wvT", [D, E], F32R, isOutput=False)
    woT = nc.declare_dram_parameter("woT", [E, D], F32R, isOutput=False)
    z = nc.declare_dram_parameter("z", [T, D], F32, isOutput=True)
    dbg = {}
    if _NC_CACHE.get("debug"):
        for nm, shp in [
            ("dqt", [128, 512]), ("dkt", [128, T]), ("dv", [128, 768]),
            ("dpt", [128, 1024]), ("dya", [96, 512]), ("dyb", [96, 512]),
            ("dysb", [128, 512]),
        ]:
            dbg[nm] = nc.declare_dram_parameter(nm, shp, F32, isOutput=True)

    with tile.TileContext(nc) as tc:
        with (
            tc.tile_pool(name="pw", bufs=8) as pw,
            tc.tile_pool(name="pwo", bufs=4) as pwo,
            tc.tile_pool(name="px", bufs=8) as px,
            tc.tile_pool(name="pkt", bufs=4) as pkt,
            tc.tile_pool(name="pqt", bufs=4) as pqt,
            tc.tile_pool(name="pv", bufs=16) as pv,
            tc.tile_pool(name="ppt", bufs=2) as ppt,
            tc.tile_pool(name="pr", bufs=2) as pr,
            tc.tile_pool(name="pysb", bufs=4) as pysb,
            tc.tile_pool(name="pzsb", bufs=1) as pzsb,
            tc.tile_pool(name="pone", bufs=1) as pone,
            tc.tile_pool(name="ps", bufs=2, space="PSUM") as ps,
            tc.tile_pool(name="pyd", bufs=2, space="PSUM") as pyd,
        ):
            # ---- weights
            wqk = []
            for dc in range(ND):
                t_ = pw.tile([128, 2 * E], F32R, tag="wqk")
                nc.sync.dma_start(t_[:], wqkT[dc * 128 : (dc + 1) * 128, :])
                wqk.append(t_)
            wv = []
            for dc in range(ND):
                t_ = pw.tile([128, E], F32R, tag="wv")
                nc.sync.dma_start(t_[:], wvT[dc * 128 : (dc + 1) * 128, :])
                wv.append(t_)
            wo = []
            for m in range(4):
                t_ = pwo.tile([128, D], F32R, tag="wo")
                nc.sync.dma_start(t_[:], woT[m * 128 : (m + 1) * 128, :])
                wo.append(t_)

            ones_f = pone.tile([128, 256], F32, tag="onef")
            nc.gpsimd.memset(ones_f[:], 1.0)

            # persistent K^T [e,t] tiles; pair m = heads 2m / 2m+1 at
            # partition rows 0:64 / 64:128
            kt = [
                pkt.tile([128, T], F32R, tag="kt", name=f"kt{i}")
                for i in range(4)
            ]
            vt = [None] * 16  # V tiles per 128-row t-block

            def emit_x_loads(tci):
                xs = []
                t0 = tci * 512
                for dc in range(ND):
                    t_ = px.tile([128, 512], F32R, tag="x", name="xs")
                    nc.sync.dma_start(
                        t_[:], xT[dc * 128 : (dc + 1) * 128, t0 : t0 + 512]
                    )
                    xs.append(t_)
                return xs

            def emit_qk_group(xs, m, tci):
                """m 0..3: Q chunk -> returns qt tile; 4..7: K chunk."""
                acc = ps.tile([128, 1024], F32, tag="st", name="acc")
                acc = acc[:, 0:512]
                for dc in range(ND):
                    nc.tensor.matmul(
                        acc[:],
                        wqk[dc][:, m * 128 : (m + 1) * 128],
                        xs[dc][:],
                        start=(dc == 0),
                        stop=(dc == ND - 1),
                    )
                if m < 4:
                    t_ = pqt.tile([128, 512], F32R, tag="qt", name="qt")
                    nc.vector.tensor_copy(t_[:], acc[:])
                    return t_
                t0 = tci * 512
                nc.vector.tensor_copy(kt[m - 4][:, t0 : t0 + 512], acc[:])
                return None

            def emit_v_group(xs, tci, ts):
                """V tile layout: [V_h(64) | ones(32)] per head, so AV
                lhsT [.., 96] slices put Y at PSUM rows 0:64 and the
                denominator at 64:96."""
                jb = 4 * tci + ts
                acc = ps.tile([128, 1024], F32, tag="st", name="vacc")
                acc = acc[:, 0:512]
                for dc in range(ND):
                    nc.tensor.matmul(
                        acc[:],
                        xs[dc][:, ts * 128 : (ts + 1) * 128],
                        wv[dc][:],
                        start=(dc == 0),
                        stop=(dc == ND - 1),
                    )
                t_ = pv.tile([128, 768], F32R, tag="v", name="vt")
                t4 = t_[:].rearrange("p (hh c) -> p hh c", hh=8)
                a4 = acc[:].rearrange("p (hh c) -> p hh c", hh=8)
                nc.vector.tensor_copy(t4[:, :, 0:64], a4[:])
                o4 = ones_f[:].rearrange("p (hh c) -> p hh c", hh=8)
                nc.vector.tensor_copy(t4[:, :, 64:96], o4[:])
                vt[jb] = t_

            # ---- prologue: chunk 0 inputs + QKV production
            xs_cur = emit_x_loads(0)
            qt_cur = [emit_qk_group(xs_cur, m, 0) for m in range(4)]
            for m in range(4, 8):
                emit_qk_group(xs_cur, m, 0)
            for ts in range(4):
                emit_v_group(xs_cur, 0, ts)

            for tc_i in range(NTC):
                if dbg and tc_i == 0:
                    nc.sync.dma_start(dbg["dqt"][:], qt_cur[0][:].bitcast(F32))
                    nc.sync.dma_start(dbg["dv"][:], vt[0][:].bitcast(F32))

                if tc_i + 1 < NTC:
                    xs_next = emit_x_loads(tc_i + 1)
                    qt_next = [None] * 4
                else:
                    xs_next = None
                    qt_next = None

                # ---- attention for i-chunk ci = tc_i
                # single pair at a time; a filler queue of next-chunk QKV
                # groups keeps the PE gapless (HAM stays warm) while ACT
                # runs the exps
                ysb_list = [None] * 4
                njb = 4 * tc_i + 4

                def emit_s(m, qtm, jb):
                    st = ps.tile([128, 1024], F32, tag="st", name="st")
                    for h in range(2):
                        nc.tensor.matmul(
                            st[:, h * 512 : h * 512 + 512],
                            kt[m][
                                h * 64 : h * 64 + 64,
                                jb * 128 : (jb + 1) * 128,
                            ],
                            qtm[h * 64 : h * 64 + 64, :],
                            start=True,
                            stop=True,
                        )
                    return st

                for m in range(4):
                    qtm = qt_cur[m]
                    ya = pyd.tile([96, 512], F32, tag="ya")
                    yb = pyd.tile([96, 512], F32, tag="yb")
                    st_next = emit_s(m, qtm, 0)
                    for jb in range(njb):
                        st = st_next
                        pt = ppt.tile([128, 1024], F32R, tag="pt", name="pt")
                        nc.scalar.activation(pt[:], st[:], EXP, scale=SCALE)
                        if jb + 1 < njb:
                            st_next = emit_s(m, qtm, jb + 1)
                        if dbg and tc_i == 0 and m == 0 and jb == 0:
                            nc.sync.dma_start(dbg["dpt"][:], pt[:].bitcast(F32))
                        if jb >= 4 * tc_i:
                            r = jb - 4 * tc_i
                            for h in range(2):
                                half = pt[:, h * 512 : h * 512 + 512]
                                nc.gpsimd.affine_select(
                                    out=half,
                                    in_=half,
                                    compare_op=mybir.AluOpType.is_ge,
                                    fill=0.0,
                                    base=-128 * r,
                                    pattern=[[1, 512]],
                                    channel_multiplier=-1,
                                )
                        first, last = (jb == 0), (jb == njb - 1)
                        nc.tensor.matmul(
                            ya[:],
                            vt[jb][:, m * 192 : m * 192 + 96],
                            pt[:, 0:512],
                            start=first,
                            stop=last,
                        )
                        nc.tensor.matmul(
                            yb[:],
                            vt[jb][:, m * 192 + 96 : m * 192 + 192],
                            pt[:, 512:1024],
                            start=first,
                            stop=last,
                        )
                    if dbg and tc_i == 0 and m == 0:
                        dya_sb = pzsb.tile(
                            [128, 1024], F32, tag="zsb", name="dya_sb"
                        )
                        nc.vector.tensor_copy(dya_sb[0:96, 0:512], ya[:])
                        nc.sync.dma_start(dbg["dya"][:], dya_sb[0:96, 0:512])
                        dyb_sb = pzsb.tile(
                            [128, 1024], F32, tag="zsb", name="dyb_sb"
                        )
                        nc.vector.tensor_copy(dyb_sb[0:96, 0:512], yb[:])
                        nc.sync.dma_start(dbg["dyb"][:], dyb_sb[0:96, 0:512])

                    rca = pr.tile([128, 512], F32, tag="rca", bufs=1)
                    nc.vector.tensor_copy(rca[64:65, :], ya[64:65, :])
                    rcb = pr.tile([128, 512], F32, tag="rcb", bufs=1)
                    nc.vector.tensor_copy(rcb[64:65, :], yb[64:65, :])
                    rc0 = pr.tile([1, 1024], F32, tag="rc0", bufs=1)
                    nc.sync.dma_start(rc0[0:1, 0:512], rca[64:65, :])
                    nc.sync.dma_start(rc0[0:1, 512:1024], rcb[64:65, :])
                    nc.vector.reciprocal_approx_fast(
                        rca[0:1, :], rc0[0:1, 0:512]
                    )
                    nc.vector.reciprocal_approx_fast(
                        rcb[0:1, :], rc0[0:1, 512:1024]
                    )
                    rba = pr.tile([128, 512], F32, tag="rba", bufs=2)
                    nc.gpsimd.partition_broadcast(rba[0:64, :], rca[0:1, :])
                    rbb = pr.tile([128, 512], F32, tag="rbb", bufs=2)
                    nc.gpsimd.partition_broadcast(rbb[0:64, :], rcb[0:1, :])
                    ytmp = pr.tile([128, 512], F32R, tag="ytmp", bufs=1)
                    nc.vector.tensor_mul(
                        ytmp[0:64, :], yb[0:64, :], rbb[0:64, :]
                    )
                    ysb = pysb.tile([128, 512], F32R, tag="ysb", name="ysb")
                    nc.vector.tensor_mul(
                        ysb[0:64, :], ya[0:64, :], rba[0:64, :]
                    )
                    nc.sync.dma_start(ysb[64:128, :], ytmp[0:64, :])
                    if dbg and tc_i == 0 and m == 0:
                        nc.sync.dma_start(dbg["dysb"][:], ysb[:].bitcast(F32))
                    ysb_list[m] = ysb

                # next chunk's QKV production fills the last pair's
                # normalize-drain before the out-projection needs it
                if xs_next is not None:
                    for mm in range(4):
                        qt_next[mm] = emit_qk_group(xs_next, mm, tc_i + 1)
                        emit_qk_group(xs_next, mm + 4, tc_i + 1)
                    for ts in range(4):
                        emit_v_group(xs_next, tc_i + 1, ts)

                # ---- out-projection for i-chunk tc_i
                for ib in range(4):
                    for fh in range(2):
                        zp = ps.tile([128, 1024], F32, tag="st", name="zp")
                        zp = zp[:, 0:512]
                        for m in range(4):
                            nc.tensor.matmul(
                                zp[:],
                                ysb_list[m][:, ib * 128 : (ib + 1) * 128],
                                wo[m][:, fh * 512 : fh * 512 + 512],
                                start=(m == 0),
                                stop=(m == 3),
                            )
                        zsb = pzsb.tile([128, 512], F32, tag="zsb", bufs=2)
                        nc.vector.tensor_copy(zsb[:], zp[:])
                        row = (4 * tc_i + ib) * 128
                        nc.sync.dma_start(
                            z[row : row + 128, fh * 512 : fh * 512 + 512],
                            zsb[:],
                        )

                qt_cur = qt_next
                xs_cur = xs_next

            if dbg:
                nc.sync.dma_start(dbg["dkt"][:], kt[0][:].bitcast(F32))

    nc.finalize()
    _NC_CACHE["nc"] = nc
    return nc


def _in_maps(x, Wqkv, Wout):
    x = np.ascontiguousarray(np.asarray(x, dtype=np.float32))
    Wqkv = np.ascontiguousarray(np.asarray(Wqkv, dtype=np.float32))
    Wout = np.ascontiguousarray(np.asarray(Wout, dtype=np.float32))
    xTs = [np.ascontiguousarray(x[b].T) for b in range(B)]
    maps = []
    for c in range(8):
        b, g = divmod(c, 2)
        qrows = Wqkv[E * g : E * g + E]
        krows = Wqkv[D + E * g : D + E * g + E]
        vrows = Wqkv[2 * D + E * g : 2 * D + E * g + E]
        maps.append(
            {
                "xT": xTs[b],
                "wqkT": np.ascontiguousarray(
                    np.concatenate([qrows, krows], axis=0).T
                ),
                "wvT": np.ascontiguousarray(vrows.T),
                "woT": np.ascontiguousarray(Wout[:, E * g : E * g + E].T),
            }
        )
    return maps


def _run(x, Wqkv, Wout, trace=False):
    from concourse.bass_utils import run_bass_kernel_spmd

    nc = build()
    res = run_bass_kernel_spmd(
        nc, _in_maps(x, Wqkv, Wout), core_ids=list(range(8)), trace=trace
    )
    out = np.empty((B, T, D), dtype=np.float32)
    for b in range(B):
        out[b] = res.results[2 * b]["z"] + res.results[2 * b + 1]["z"]
    return out, res


def kernel(x, Wqkv, Wout):
    out, _ = _run(x, Wqkv, Wout, trace=False)
    return out



# revision 3
# speedup vs baseline: 1.1541x; 1.1541x over previous
"""Multi-head causal attention on 8 TRN2 NeuronCores.

Problem: x[4,2048,1024] @ Wqkv.T -> 16-head causal attention -> @ Wout.T.

Sharding: core c handles batch b=c//2, head-group g=c%2 (8 heads of 64).
Each core computes qkv for its (batch, head-group) slice, causal attention,
and a partial out-projection over its 512 columns of Wout's input dim.
Host sums the two partials per batch (the all-reduce of the hint).

Per-core layouts (host pre-transposes so every matmul contraction dim lands
on SBUF partitions):
  xT   [1024 d, 2048 t]      wqkT [1024 d, 1024 (q|k)e]
  wvT  [1024 d,  512 e]      woT  [ 512 e, 1024 f]
All tensors are fp16 (PSUM accumulation stays fp32): same 1-row/cycle PE
rate as fp32r but FWL halves LDWEIGHTS, DMA bytes halve, and the PE power
draw drops below the SW-throttle threshold that cost fp32r ~75us of K=4/8
clock-gating.  Simulated end-to-end fp16 error: 5.7e-4 rel (gate: 2e-2).

Emission is software-pipelined to keep the PE dense: the QKV-production
matmul groups for t-chunk tc+1 are interleaved into the attention phase of
chunk tc as PE filler; S^T for jb+1 is emitted before AV of jb so the PE
never waits on the ACT exp.  S head-pairs run concurrently on row-groups
0:63 / 64:127 (auto tile_position from the 64-partition APs).
"""

import sys

sys.path.insert(0, "/opt/trn_rl_repo")

import numpy as np

B, T, D, H = 4, 2048, 1024, 16
E = 512  # per-core head width (8 heads x 64)
ND = 8  # d chunks of 128
NTC = 4  # t chunks of 512
SCALE = 0.125  # 1/sqrt(64)

_NC_CACHE = {}


def build():
    if "nc" in _NC_CACHE:
        return _NC_CACHE["nc"]
    import concourse.bacc as bacc
    import concourse.mybir as mybir
    import concourse.tile as tile

    F32 = mybir.dt.float32
    F16 = mybir.dt.float16
    EXP = mybir.ActivationFunctionType.Exp

    nc = bacc.Bacc("TRN2", target_bir_lowering=False, debug=False, num_devices=8)
    xT = nc.declare_dram_parameter("xT", [D, T], F16, isOutput=False)
    wqkT = nc.declare_dram_parameter("wqkT", [D, 2 * E], F16, isOutput=False)
    wvT = nc.declare_dram_parameter("wvT", [D, E], F16, isOutput=False)
    woT = nc.declare_dram_parameter("woT", [E, D], F16, isOutput=False)
    z = nc.declare_dram_parameter("z", [T, D], F16, isOutput=True)

    with tile.TileContext(nc) as tc:
        with (
            tc.tile_pool(name="pw", bufs=8) as pw,
            tc.tile_pool(name="pwo", bufs=4) as pwo,
            tc.tile_pool(name="px", bufs=8) as px,
            tc.tile_pool(name="pkt", bufs=4) as pkt,
            tc.tile_pool(name="pqt", bufs=4) as pqt,
            tc.tile_pool(name="pv", bufs=16) as pv,
            tc.tile_pool(name="ppt", bufs=2) as ppt,
            tc.tile_pool(name="pr", bufs=2) as pr,
            tc.tile_pool(name="pysb", bufs=4) as pysb,
            tc.tile_pool(name="pzsb", bufs=1) as pzsb,
            tc.tile_pool(name="pone", bufs=1) as pone,
            tc.tile_pool(name="ps", bufs=2, space="PSUM") as ps,
            tc.tile_pool(name="pyd", bufs=2, space="PSUM") as pyd,
        ):
            # ---- first x chunk + qk weights interleaved so the first QKV
            # matmul is gated on ~1MB of DMA, not the whole 4.5MB weight set
            wqk = []
            xs0 = []
            for dc in range(ND):
                x_ = px.tile([128, 512], F16, tag="x", name="xs")
                nc.sync.dma_start(x_[:], xT[dc * 128 : (dc + 1) * 128, 0:512])
                xs0.append(x_)
                t_ = pw.tile([128, 2 * E], F16, tag="wqk")
                nc.sync.dma_start(t_[:], wqkT[dc * 128 : (dc + 1) * 128, :])
                wqk.append(t_)
            wv = []
            for dc in range(ND):
                t_ = pw.tile([128, E], F16, tag="wv")
                nc.sync.dma_start(t_[:], wvT[dc * 128 : (dc + 1) * 128, :])
                wv.append(t_)
            wo = []
            for m in range(4):
                t_ = pwo.tile([128, D], F16, tag="wo")
                nc.sync.dma_start(t_[:], woT[m * 128 : (m + 1) * 128, :])
                wo.append(t_)

            # per-head filler block for the AV stationary: [ones(32)|zeros(32)]
            ones_f = pone.tile([128, 512], F16, tag="onef")
            of4 = ones_f[:].rearrange("p (hh c) -> p hh c", hh=8)
            nc.gpsimd.memset(of4[:, :, 0:32], 1.0)
            nc.gpsimd.memset(of4[:, :, 32:64], 0.0)

            # persistent K^T [e,t] tiles; pair m = heads 2m / 2m+1 at
            # partition rows 0:64 / 64:128
            kt = [
                pkt.tile([128, T], F16, tag="kt", name=f"kt{i}")
                for i in range(4)
            ]
            vt = [None] * 16  # V tiles per 128-row t-block

            def emit_x_loads(tci):
                xs = []
                t0 = tci * 512
                for dc in range(ND):
                    t_ = px.tile([128, 512], F16, tag="x", name="xs")
                    nc.sync.dma_start(
                        t_[:], xT[dc * 128 : (dc + 1) * 128, t0 : t0 + 512]
                    )
                    xs.append(t_)
                return xs

            def emit_qk_group(xs, m, tci):
                """m 0..3: Q chunk -> returns qt tile; 4..7: K chunk."""
                acc = ps.tile([128, 1024], F32, tag="st", name="acc")
                acc = acc[:, 0:512]
                for dc in range(ND):
                    nc.tensor.matmul(
                        acc[:],
                        wqk[dc][:, m * 128 : (m + 1) * 128],
                        xs[dc][:],
                        start=(dc == 0),
                        stop=(dc == ND - 1),
                    )
                if m < 4:
                    t_ = pqt.tile([128, 512], F16, tag="qt", name="qt")
                    nc.vector.tensor_copy(t_[:], acc[:])
                    return t_
                t0 = tci * 512
                nc.vector.tensor_copy(kt[m - 4][:, t0 : t0 + 512], acc[:])
                return None

            def emit_v_group(xs, tci, ts):
                """V tile layout per head: [V_h(64) | ones(32) | zeros(32)]
                -> AV lhsT slices are 128 cols (FWL) and put Y at PSUM rows
                0:64, the denominator at 64:96, zeros at 96:128."""
                jb = 4 * tci + ts
                acc = ps.tile([128, 1024], F32, tag="st", name="vacc")
                acc = acc[:, 0:512]
                for dc in range(ND):
                    nc.tensor.matmul(
                        acc[:],
                        xs[dc][:, ts * 128 : (ts + 1) * 128],
                        wv[dc][:],
                        start=(dc == 0),
                        stop=(dc == ND - 1),
                    )
                t_ = pv.tile([128, 1024], F16, tag="v", name="vt")
                t4 = t_[:].rearrange("p (hh c) -> p hh c", hh=8)
                a4 = acc[:].rearrange("p (hh c) -> p hh c", hh=8)
                nc.vector.tensor_copy(t4[:, :, 0:64], a4[:])
                nc.vector.tensor_copy(t4[:, :, 64:128], of4[:])
                vt[jb] = t_

            # ---- prologue: chunk 0 QKV production
            xs_cur = xs0
            qt_cur = [emit_qk_group(xs_cur, m, 0) for m in range(4)]
            for m in range(4, 8):
                emit_qk_group(xs_cur, m, 0)
            for ts in range(4):
                emit_v_group(xs_cur, 0, ts)

            for tc_i in range(NTC):
                if tc_i + 1 < NTC:
                    xs_next = emit_x_loads(tc_i + 1)
                    qt_next = [None] * 4
                else:
                    xs_next = None
                    qt_next = None

                # ---- attention for i-chunk ci = tc_i
                ysb_list = [None] * 4
                njb = 4 * tc_i + 4

                def emit_s(m, qtm, jb):
                    st = ps.tile([128, 1024], F32, tag="st", name="st")
                    for h in range(2):
                        nc.tensor.matmul(
                            st[:, h * 512 : h * 512 + 512],
                            kt[m][
                                h * 64 : h * 64 + 64,
                                jb * 128 : (jb + 1) * 128,
                            ],
                            qtm[h * 64 : h * 64 + 64, :],
                            start=True,
                            stop=True,
                        )
                    return st

                for m in range(4):
                    qtm = qt_cur[m]
                    ya = pyd.tile([128, 512], F32, tag="ya")
                    yb = pyd.tile([128, 512], F32, tag="yb")
                    st_next = emit_s(m, qtm, 0)
                    for jb in range(njb):
                        st = st_next
                        pt = ppt.tile([128, 1024], F16, tag="pt", name="pt")
                        nc.scalar.activation(pt[:], st[:], EXP, scale=SCALE)
                        if jb + 1 < njb:
                            st_next = emit_s(m, qtm, jb + 1)
                        if jb >= 4 * tc_i:
                            r = jb - 4 * tc_i
                            for h in range(2):
                                half = pt[:, h * 512 : h * 512 + 512]
                                nc.gpsimd.affine_select(
                                    out=half,
                                    in_=half,
                                    compare_op=mybir.AluOpType.is_ge,
                                    fill=0.0,
                                    base=-128 * r,
                                    pattern=[[1, 512]],
                                    channel_multiplier=-1,
                                )
                        first, last = (jb == 0), (jb == njb - 1)
                        nc.tensor.matmul(
                            ya[:],
                            vt[jb][:, m * 256 : m * 256 + 128],
                            pt[:, 0:512],
                            start=first,
                            stop=last,
                        )
                        nc.tensor.matmul(
                            yb[:],
                            vt[jb][:, m * 256 + 128 : m * 256 + 256],
                            pt[:, 512:1024],
                            start=first,
                            stop=last,
                        )

                    # normalize: 1/den rows live at PSUM row 64 of ya/yb
                    rca = pr.tile([128, 512], F32, tag="rca", bufs=1)
                    nc.vector.tensor_copy(rca[64:65, :], ya[64:65, :])
                    rcb = pr.tile([128, 512], F32, tag="rcb", bufs=1)
                    nc.vector.tensor_copy(rcb[64:65, :], yb[64:65, :])
                    rc0 = pr.tile([1, 1024], F32, tag="rc0", bufs=1)
                    nc.sync.dma_start(rc0[0:1, 0:512], rca[64:65, :])
                    nc.sync.dma_start(rc0[0:1, 512:1024], rcb[64:65, :])
                    nc.vector.reciprocal_approx_fast(
                        rca[0:1, :], rc0[0:1, 0:512]
                    )
                    nc.vector.reciprocal_approx_fast(
                        rcb[0:1, :], rc0[0:1, 512:1024]
                    )
                    rba = pr.tile([128, 512], F32, tag="rba", bufs=2)
                    nc.gpsimd.partition_broadcast(rba[0:64, :], rca[0:1, :])
                    rbb = pr.tile([128, 512], F32, tag="rbb", bufs=2)
                    nc.gpsimd.partition_broadcast(rbb[0:64, :], rcb[0:1, :])
                    ytmp = pr.tile([128, 512], F16, tag="ytmp", bufs=1)
                    nc.vector.tensor_mul(
                        ytmp[0:64, :], yb[0:64, :], rbb[0:64, :]
                    )
                    ysb = pysb.tile([128, 512], F16, tag="ysb", name="ysb")
                    nc.vector.tensor_mul(
                        ysb[0:64, :], ya[0:64, :], rba[0:64, :]
                    )
                    nc.sync.dma_start(ysb[64:128, :], ytmp[0:64, :])
                    ysb_list[m] = ysb

                # next chunk's QKV production fills the last pair's
                # normalize-drain before the out-projection needs it
                if xs_next is not None:
                    for mm in range(4):
                        qt_next[mm] = emit_qk_group(xs_next, mm, tc_i + 1)
                        emit_qk_group(xs_next, mm + 4, tc_i + 1)
                    for ts in range(4):
                        emit_v_group(xs_next, tc_i + 1, ts)

                # ---- out-projection for i-chunk tc_i
                for ib in range(4):
                    for fh in range(2):
                        zp = ps.tile([128, 1024], F32, tag="st", name="zp")
                        zp = zp[:, 0:512]
                        for m in range(4):
                            nc.tensor.matmul(
                                zp[:],
                                ysb_list[m][:, ib * 128 : (ib + 1) * 128],
                                wo[m][:, fh * 512 : fh * 512 + 512],
                                start=(m == 0),
                                stop=(m == 3),
                            )
                        zsb = pzsb.tile([128, 512], F16, tag="zsb", bufs=2)
                        nc.vector.tensor_copy(zsb[:], zp[:])
                        row = (4 * tc_i + ib) * 128
                        nc.sync.dma_start(
                            z[row : row + 128, fh * 512 : fh * 512 + 512],
                            zsb[:],
                        )

                qt_cur = qt_next
                xs_cur = xs_next

    nc.finalize()
    _NC_CACHE["nc"] = nc
    return nc


def _in_maps(x, Wqkv, Wout):
    x = np.asarray(x, dtype=np.float32)
    Wqkv = np.asarray(Wqkv, dtype=np.float32)
    Wout = np.asarray(Wout, dtype=np.float32)
    xTs = [np.ascontiguousarray(x[b].T.astype(np.float16)) for b in range(B)]
    maps = []
    for c in range(8):
        b, g = divmod(c, 2)
        qrows = Wqkv[E * g : E * g + E]
        krows = Wqkv[D + E * g : D + E * g + E]
        vrows = Wqkv[2 * D + E * g : 2 * D + E * g + E]
        maps.append(
            {
                "xT": xTs[b],
                "wqkT": np.ascontiguousarray(
                    np.concatenate([qrows, krows], axis=0).T.astype(np.float16)
                ),
                "wvT": np.ascontiguousarray(vrows.T.astype(np.float16)),
                "woT": np.ascontiguousarray(
                    Wout[:, E * g : E * g + E].T.astype(np.float16)
                ),
            }
        )
    return maps


def _run(x, Wqkv, Wout, trace=False):
    from concourse.bass_utils import run_bass_kernel_spmd

    nc = build()
    res = run_bass_kernel_spmd(
        nc, _in_maps(x, Wqkv, Wout), core_ids=list(range(8)), trace=trace
    )
    out = np.empty((B, T, D), dtype=np.float32)
    for b in range(B):
        out[b] = res.results[2 * b]["z"].astype(np.float32) + res.results[
            2 * b + 1
        ]["z"].astype(np.float32)
    return out, res


def kernel(x, Wqkv, Wout):
    out, _ = _run(x, Wqkv, Wout, trace=False)
    return out


# revision 5
# speedup vs baseline: 1.1626x; 1.0074x over previous
"""Multi-head causal attention on 8 TRN2 NeuronCores.

Problem: x[4,2048,1024] @ Wqkv.T -> 16-head causal attention -> @ Wout.T.

Sharding: core c handles batch b=c//2, head-group g=c%2 (8 heads of 64).
Each core computes qkv for its (batch, head-group) slice, causal attention,
and a partial out-projection over its 512 columns of Wout's input dim.
Host sums the two partials per batch (the all-reduce of the hint).

Per-core layouts (host pre-transposes so every matmul contraction dim lands
on SBUF partitions):
  xT   [1024 d, 2048 t]      wqkT [1024 d, 1024 (q|k)e]
  wvT  [1024 d,  512 e]      woT  [ 512 e, 1024 f]
All tensors are fp16 (PSUM accumulation stays fp32): same 1-row/cycle PE
rate as fp32r but FWL halves LDWEIGHTS, DMA bytes halve, and the PE power
draw drops below the SW-throttle threshold that cost fp32r ~75us of K=4/8
clock-gating.  Simulated end-to-end fp16 error: 5.7e-4 rel (gate: 2e-2).

Emission is software-pipelined to keep the PE dense: the QKV-production
matmul groups for t-chunk tc+1 are interleaved into the attention phase of
chunk tc as PE filler; S^T for jb+1 is emitted before AV of jb so the PE
never waits on the ACT exp.  S head-pairs run concurrently on row-groups
0:63 / 64:127 (auto tile_position from the 64-partition APs).
"""

import sys

sys.path.insert(0, "/opt/trn_rl_repo")

import numpy as np

B, T, D, H = 4, 2048, 1024, 16
E = 512  # per-core head width (8 heads x 64)
ND = 8  # d chunks of 128
NTC = 4  # t chunks of 512
SCALE = 0.125  # 1/sqrt(64)

_NC_CACHE = {}


def build():
    if "nc" in _NC_CACHE:
        return _NC_CACHE["nc"]
    import concourse.bacc as bacc
    import concourse.mybir as mybir
    import concourse.tile as tile

    F32 = mybir.dt.float32
    F16 = mybir.dt.float16
    EXP = mybir.ActivationFunctionType.Exp

    nc = bacc.Bacc("TRN2", target_bir_lowering=False, debug=False, num_devices=8)
    xT = nc.declare_dram_parameter("xT", [D, T], F16, isOutput=False)
    wqkT = nc.declare_dram_parameter("wqkT", [D, 2 * E], F16, isOutput=False)
    wvT = nc.declare_dram_parameter("wvT", [D, E], F16, isOutput=False)
    woT = nc.declare_dram_parameter("woT", [E, D], F16, isOutput=False)
    z = nc.declare_dram_parameter("z", [T, D], F16, isOutput=True)

    with tile.TileContext(nc) as tc:
        with (
            tc.tile_pool(name="pw", bufs=8) as pw,
            tc.tile_pool(name="pwo", bufs=4) as pwo,
            tc.tile_pool(name="px", bufs=8) as px,
            tc.tile_pool(name="pkt", bufs=4) as pkt,
            tc.tile_pool(name="pqt", bufs=4) as pqt,
            tc.tile_pool(name="pv", bufs=16) as pv,
            tc.tile_pool(name="ppt", bufs=2) as ppt,
            tc.tile_pool(name="pr", bufs=2) as pr,
            tc.tile_pool(name="pysb", bufs=4) as pysb,
            tc.tile_pool(name="pzsb", bufs=1) as pzsb,
            tc.tile_pool(name="pone", bufs=1) as pone,
            tc.tile_pool(name="ps", bufs=2, space="PSUM") as ps,
            tc.tile_pool(name="pyd", bufs=2, space="PSUM") as pyd,
        ):
            # ---- first x chunk + qk weights interleaved so the first QKV
            # matmul is gated on ~1MB of DMA, not the whole 4.5MB weight set
            wqk = []
            xs0 = []
            for dc in range(ND):
                x_ = px.tile([128, 512], F16, tag="x", name="xs")
                nc.sync.dma_start(x_[:], xT[dc * 128 : (dc + 1) * 128, 0:512])
                xs0.append(x_)
                t_ = pw.tile([128, 2 * E], F16, tag="wqk")
                nc.sync.dma_start(t_[:], wqkT[dc * 128 : (dc + 1) * 128, :])
                wqk.append(t_)
            wv = []
            for dc in range(ND):
                t_ = pw.tile([128, E], F16, tag="wv")
                nc.sync.dma_start(t_[:], wvT[dc * 128 : (dc + 1) * 128, :])
                wv.append(t_)
            wo = []
            for m in range(4):
                t_ = pwo.tile([128, D], F16, tag="wo")
                nc.sync.dma_start(t_[:], woT[m * 128 : (m + 1) * 128, :])
                wo.append(t_)

            # per-head filler block for the AV stationary: [ones(32)|zeros(32)]
            ones_f = pone.tile([128, 512], F16, tag="onef")
            of4 = ones_f[:].rearrange("p (hh c) -> p hh c", hh=8)
            nc.gpsimd.memset(of4[:, :, 0:32], 1.0)
            nc.gpsimd.memset(of4[:, :, 32:64], 0.0)

            # persistent K^T [e,t] tiles; pair m = heads 2m / 2m+1 at
            # partition rows 0:64 / 64:128
            kt = [
                pkt.tile([128, T], F16, tag="kt", name=f"kt{i}")
                for i in range(4)
            ]
            vt = [None] * 16  # V tiles per 128-row t-block

            def emit_x_loads(tci):
                xs = []
                t0 = tci * 512
                for dc in range(ND):
                    t_ = px.tile([128, 512], F16, tag="x", name="xs")
                    nc.sync.dma_start(
                        t_[:], xT[dc * 128 : (dc + 1) * 128, t0 : t0 + 512]
                    )
                    xs.append(t_)
                return xs

            def emit_qk_group(xs, m, tci):
                """m 0..3: Q chunk -> returns qt tile; 4..7: K chunk."""
                acc = ps.tile([128, 1024], F32, tag="st", name="acc")
                acc = acc[:, 0:512]
                for dc in range(ND):
                    nc.tensor.matmul(
                        acc[:],
                        wqk[dc][:, m * 128 : (m + 1) * 128],
                        xs[dc][:],
                        start=(dc == 0),
                        stop=(dc == ND - 1),
                    )
                if m < 4:
                    t_ = pqt.tile([128, 512], F16, tag="qt", name="qt")
                    nc.vector.tensor_copy(t_[:], acc[:])
                    return t_
                t0 = tci * 512
                nc.vector.tensor_copy(kt[m - 4][:, t0 : t0 + 512], acc[:])
                return None

            def emit_v_group(xs, tci, ts):
                """V tile layout per head: [V_h(64) | ones(32) | zeros(32)]
                -> AV lhsT slices are 128 cols (FWL) and put Y at PSUM rows
                0:64, the denominator at 64:96, zeros at 96:128."""
                jb = 4 * tci + ts
                acc = ps.tile([128, 1024], F32, tag="st", name="vacc")
                acc = acc[:, 0:512]
                for dc in range(ND):
                    nc.tensor.matmul(
                        acc[:],
                        xs[dc][:, ts * 128 : (ts + 1) * 128],
                        wv[dc][:],
                        start=(dc == 0),
                        stop=(dc == ND - 1),
                    )
                t_ = pv.tile([128, 1024], F16, tag="v", name="vt")
                t4 = t_[:].rearrange("p (hh c) -> p hh c", hh=8)
                a4 = acc[:].rearrange("p (hh c) -> p hh c", hh=8)
                nc.vector.tensor_copy(t4[:, :, 0:64], a4[:])
                nc.vector.tensor_copy(t4[:, :, 64:128], of4[:])
                vt[jb] = t_

            # ---- prologue: chunk 0 QKV production
            xs_cur = xs0
            qt_cur = [emit_qk_group(xs_cur, m, 0) for m in range(4)]
            for m in range(4, 8):
                emit_qk_group(xs_cur, m, 0)
            for ts in range(4):
                emit_v_group(xs_cur, 0, ts)

            for tc_i in range(NTC):
                if tc_i + 1 < NTC:
                    xs_next = emit_x_loads(tc_i + 1)
                    qt_next = [None] * 4
                else:
                    xs_next = None
                    qt_next = None

                # ---- attention for i-chunk ci = tc_i
                ysb_list = [None] * 4
                njb = 4 * tc_i + 4

                def emit_s(m, qtm, jb):
                    st = ps.tile([128, 1024], F32, tag="st", name="st")
                    for h in range(2):
                        nc.tensor.matmul(
                            st[:, h * 512 : h * 512 + 512],
                            kt[m][
                                h * 64 : h * 64 + 64,
                                jb * 128 : (jb + 1) * 128,
                            ],
                            qtm[h * 64 : h * 64 + 64, :],
                            start=True,
                            stop=True,
                        )
                    return st

                for m in range(4):
                    qtm = qt_cur[m]
                    ya = pyd.tile([128, 512], F32, tag="ya")
                    yb = pyd.tile([128, 512], F32, tag="yb")
                    st_next = emit_s(m, qtm, 0)
                    for jb in range(njb):
                        st = st_next
                        pt = ppt.tile([128, 1024], F16, tag="pt", name="pt")
                        nc.scalar.activation(pt[:], st[:], EXP, scale=SCALE)
                        if jb + 1 < njb:
                            st_next = emit_s(m, qtm, jb + 1)
                        if jb >= 4 * tc_i:
                            # causal mask: the q<j cutoff lies inside a
                            # single 128-col window per head; memset the
                            # all-masked prefix, affine_select the window
                            r = jb - 4 * tc_i
                            pt4 = pt[:].rearrange("p (h c) -> p h c", h=2)
                            if r:
                                nc.gpsimd.memset(pt4[:, :, 0 : 128 * r], 0.0)
                            win = pt4[:, :, 128 * r : 128 * r + 128]
                            nc.gpsimd.affine_select(
                                out=win,
                                in_=win,
                                compare_op=mybir.AluOpType.is_ge,
                                fill=0.0,
                                base=0,
                                pattern=[[0, 2], [1, 128]],
                                channel_multiplier=-1,
                            )
                        first, last = (jb == 0), (jb == njb - 1)
                        nc.tensor.matmul(
                            ya[:],
                            vt[jb][:, m * 256 : m * 256 + 128],
                            pt[:, 0:512],
                            start=first,
                            stop=last,
                        )
                        nc.tensor.matmul(
                            yb[:],
                            vt[jb][:, m * 256 + 128 : m * 256 + 256],
                            pt[:, 512:1024],
                            start=first,
                            stop=last,
                        )

                    # normalize: 1/den rows live at PSUM row 64 of ya/yb
                    rca = pr.tile([128, 512], F32, tag="rca", bufs=1)
                    nc.vector.tensor_copy(rca[64:65, :], ya[64:65, :])
                    rcb = pr.tile([128, 512], F32, tag="rcb", bufs=1)
                    nc.vector.tensor_copy(rcb[64:65, :], yb[64:65, :])
                    rc0 = pr.tile([1, 1024], F32, tag="rc0", bufs=1)
                    nc.sync.dma_start(rc0[0:1, 0:512], rca[64:65, :])
                    nc.sync.dma_start(rc0[0:1, 512:1024], rcb[64:65, :])
                    nc.vector.reciprocal_approx_fast(
                        rca[0:1, :], rc0[0:1, 0:512]
                    )
                    nc.vector.reciprocal_approx_fast(
                        rcb[0:1, :], rc0[0:1, 512:1024]
                    )
                    rba = pr.tile([128, 512], F32, tag="rba", bufs=2)
                    nc.gpsimd.partition_broadcast(rba[0:64, :], rca[0:1, :])
                    rbb = pr.tile([128, 512], F32, tag="rbb", bufs=2)
                    nc.gpsimd.partition_broadcast(rbb[0:64, :], rcb[0:1, :])
                    ytmp = pr.tile([128, 512], F16, tag="ytmp", bufs=1)
                    nc.vector.tensor_mul(
                        ytmp[0:64, :], yb[0:64, :], rbb[0:64, :]
                    )
                    ysb = pysb.tile([128, 512], F16, tag="ysb", name="ysb")
                    nc.vector.tensor_mul(
                        ysb[0:64, :], ya[0:64, :], rba[0:64, :]
                    )
                    nc.sync.dma_start(ysb[64:128, :], ytmp[0:64, :])
                    ysb_list[m] = ysb

                    # next chunk's QKV production, interleaved per pair:
                    # the in-order PE queue chews these while the attention
                    # phase is ACT-bound and while pair m's normalize chain
                    # drains (qt_cur[m] was just released by its last S)
                    if xs_next is not None:
                        qt_next[m] = emit_qk_group(xs_next, m, tc_i + 1)
                        emit_qk_group(xs_next, m + 4, tc_i + 1)
                        emit_v_group(xs_next, tc_i + 1, m)

                # ---- out-projection for i-chunk tc_i
                for ib in range(4):
                    for fh in range(2):
                        zp = ps.tile([128, 1024], F32, tag="st", name="zp")
                        zp = zp[:, 0:512]
                        for m in range(4):
                            nc.tensor.matmul(
                                zp[:],
                                ysb_list[m][:, ib * 128 : (ib + 1) * 128],
                                wo[m][:, fh * 512 : fh * 512 + 512],
                                start=(m == 0),
                                stop=(m == 3),
                            )
                        zsb = pzsb.tile([128, 512], F16, tag="zsb", bufs=2)
                        nc.vector.tensor_copy(zsb[:], zp[:])
                        row = (4 * tc_i + ib) * 128
                        nc.sync.dma_start(
                            z[row : row + 128, fh * 512 : fh * 512 + 512],
                            zsb[:],
                        )

                qt_cur = qt_next
                xs_cur = xs_next

    nc.finalize()
    _NC_CACHE["nc"] = nc
    return nc


def _in_maps(x, Wqkv, Wout):
    x = np.asarray(x, dtype=np.float32)
    Wqkv = np.asarray(Wqkv, dtype=np.float32)
    Wout = np.asarray(Wout, dtype=np.float32)
    xTs = [np.ascontiguousarray(x[b].T.astype(np.float16)) for b in range(B)]
    maps = []
    for c in range(8):
        b, g = divmod(c, 2)
        qrows = Wqkv[E * g : E * g + E]
        krows = Wqkv[D + E * g : D + E * g + E]
        vrows = Wqkv[2 * D + E * g : 2 * D + E * g + E]
        maps.append(
            {
                "xT": xTs[b],
                "wqkT": np.ascontiguousarray(
                    np.concatenate([qrows, krows], axis=0).T.astype(np.float16)
                ),
                "wvT": np.ascontiguousarray(vrows.T.astype(np.float16)),
                "woT": np.ascontiguousarray(
                    Wout[:, E * g : E * g + E].T.astype(np.float16)
                ),
            }
        )
    return maps


def _run(x, Wqkv, Wout, trace=False):
    from concourse.bass_utils import run_bass_kernel_spmd

    nc = build()
    res = run_bass_kernel_spmd(
        nc, _in_maps(x, Wqkv, Wout), core_ids=list(range(8)), trace=trace
    )
    out = np.empty((B, T, D), dtype=np.float32)
    for b in range(B):
        out[b] = res.results[2 * b]["z"].astype(np.float32) + res.results[
            2 * b + 1
        ]["z"].astype(np.float32)
    return out, res


def kernel(x, Wqkv, Wout):
    out, _ = _run(x, Wqkv, Wout, trace=False)
    return out


# revision 10
# speedup vs baseline: 1.2054x; 1.0368x over previous
"""Multi-head causal attention on 8 TRN2 NeuronCores.

Problem: x[4,2048,1024] @ Wqkv.T -> 16-head causal attention -> @ Wout.T.

Sharding: core c handles batch b=c//2, head-group g=c%2 (8 heads of 64).
Each core computes qkv for its (batch, head-group) slice, causal attention,
and a partial out-projection over its 512 columns of Wout's input dim.
Host sums the two partials per batch (the all-reduce of the hint).

Per-core layouts (host pre-transposes so every matmul contraction dim lands
on SBUF partitions):
  xT   [1024 d, 2048 t]      wqkT [1024 d, 1024 (q|k)e]
  wvT  [1024 d,  512 e]      woT  [ 512 e, 1024 f]
All tensors are fp16 (PSUM accumulation stays fp32): same 1-row/cycle PE
rate as fp32r but FWL halves LDWEIGHTS, DMA bytes halve, and the PE power
draw drops below the SW-throttle threshold that cost fp32r ~75us of K=4/8
clock-gating.  Simulated end-to-end fp16 error: 5.7e-4 rel (gate: 2e-2).

Emission is software-pipelined to keep the PE dense: the QKV-production
matmul groups for t-chunk tc+1 are interleaved into the attention phase of
chunk tc as PE filler; S^T for jb+1 is emitted before AV of jb so the PE
never waits on the ACT exp.  S head-pairs run concurrently on row-groups
0:63 / 64:127 (auto tile_position from the 64-partition APs).
"""

import sys

sys.path.insert(0, "/opt/trn_rl_repo")

import numpy as np

B, T, D, H = 4, 2048, 1024, 16
E = 512  # per-core head width (8 heads x 64)
ND = 8  # d chunks of 128
NTC = 4  # t chunks of 512
SCALE = 0.125  # 1/sqrt(64)

_NC_CACHE = {}


def build():
    if "nc" in _NC_CACHE:
        return _NC_CACHE["nc"]
    import concourse.bacc as bacc
    import concourse.mybir as mybir
    import concourse.tile as tile

    F32 = mybir.dt.float32
    F16 = mybir.dt.float16
    EXP = mybir.ActivationFunctionType.Exp

    nc = bacc.Bacc("TRN2", target_bir_lowering=False, debug=False, num_devices=8)
    xT = nc.declare_dram_parameter("xT", [D, T], F16, isOutput=False)
    wqkT = nc.declare_dram_parameter("wqkT", [D, 2 * E], F16, isOutput=False)
    wvT = nc.declare_dram_parameter("wvT", [D, E], F16, isOutput=False)
    woT = nc.declare_dram_parameter("woT", [E, D], F16, isOutput=False)
    z = nc.declare_dram_parameter("z", [T, D], F16, isOutput=True)

    with tile.TileContext(nc) as tc:
        with (
            tc.tile_pool(name="pw", bufs=8) as pw,
            tc.tile_pool(name="pwo", bufs=4) as pwo,
            tc.tile_pool(name="px", bufs=8) as px,
            tc.tile_pool(name="pkt", bufs=4) as pkt,
            tc.tile_pool(name="pqt", bufs=4) as pqt,
            tc.tile_pool(name="pv", bufs=16) as pv,
            tc.tile_pool(name="ppt", bufs=2) as ppt,
            tc.tile_pool(name="pr", bufs=2) as pr,
            tc.tile_pool(name="pysb", bufs=4) as pysb,
            tc.tile_pool(name="pzsb", bufs=1) as pzsb,
            tc.tile_pool(name="pone", bufs=1) as pone,
            tc.tile_pool(name="ps", bufs=2, space="PSUM") as ps,
            tc.tile_pool(name="pyd", bufs=2, space="PSUM") as pyd,
        ):
            # ---- first x chunk + qk weights interleaved so the first QKV
            # matmul is gated on ~1MB of DMA, not the whole 4.5MB weight set
            wqk = []
            xs0 = []
            for dc in range(ND):
                x_ = px.tile([128, 512], F16, tag="x", name="xs")
                nc.sync.dma_start(x_[:], xT[dc * 128 : (dc + 1) * 128, 0:512])
                xs0.append(x_)
                t_ = pw.tile([128, 2 * E], F16, tag="wqk")
                nc.sync.dma_start(t_[:], wqkT[dc * 128 : (dc + 1) * 128, :])
                wqk.append(t_)
            wv = []
            for dc in range(ND):
                t_ = pw.tile([128, E], F16, tag="wv")
                nc.sync.dma_start(t_[:], wvT[dc * 128 : (dc + 1) * 128, :])
                wv.append(t_)
            wo = []
            for m in range(4):
                t_ = pwo.tile([128, D], F16, tag="wo")
                nc.sync.dma_start(t_[:], woT[m * 128 : (m + 1) * 128, :])
                wo.append(t_)

            # per-head filler block for the AV stationary: [ones(32)|zeros(32)]
            ones_f = pone.tile([128, 512], F16, tag="onef")
            of4 = ones_f[:].rearrange("p (hh c) -> p hh c", hh=8)
            nc.gpsimd.memset(of4[:, :, 0:32], 1.0)
            nc.gpsimd.memset(of4[:, :, 32:64], 0.0)

            # persistent K^T [e,t] tiles; pair m = heads 2m / 2m+1 at
            # partition rows 0:64 / 64:128
            kt = [
                pkt.tile([128, T], F16, tag="kt", name=f"kt{i}")
                for i in range(4)
            ]
            vt = [None] * 16  # V tiles per 128-row t-block

            def emit_x_loads(tci):
                xs = []
                t0 = tci * 512
                for dc in range(ND):
                    t_ = px.tile([128, 512], F16, tag="x", name="xs")
                    nc.sync.dma_start(
                        t_[:], xT[dc * 128 : (dc + 1) * 128, t0 : t0 + 512]
                    )
                    xs.append(t_)
                return xs

            def emit_qk_group(xs, m, tci):
                """m 0..3: Q chunk -> returns qt tile; 4..7: K chunk."""
                acc = ps.tile([128, 1024], F32, tag="st", name="acc")
                acc = acc[:, 0:512]
                for dc in range(ND):
                    nc.tensor.matmul(
                        acc[:],
                        wqk[dc][:, m * 128 : (m + 1) * 128],
                        xs[dc][:],
                        start=(dc == 0),
                        stop=(dc == ND - 1),
                    )
                if m < 4:
                    t_ = pqt.tile([128, 512], F16, tag="qt", name="qt")
                    nc.vector.tensor_copy(t_[:], acc[:])
                    return t_
                t0 = tci * 512
                nc.vector.tensor_copy(kt[m - 4][:, t0 : t0 + 512], acc[:])
                return None

            def emit_v_group(xs, tci, ts):
                """V tile layout per head: [V_h(64) | ones(32) | zeros(32)]
                -> AV lhsT slices are 128 cols (FWL) and put Y at PSUM rows
                0:64, the denominator at 64:96, zeros at 96:128."""
                jb = 4 * tci + ts
                acc = ps.tile([128, 1024], F32, tag="st", name="vacc")
                acc = acc[:, 0:512]
                for dc in range(ND):
                    nc.tensor.matmul(
                        acc[:],
                        xs[dc][:, ts * 128 : (ts + 1) * 128],
                        wv[dc][:],
                        start=(dc == 0),
                        stop=(dc == ND - 1),
                    )
                t_ = pv.tile([128, 1024], F16, tag="v", name="vt")
                t4 = t_[:].rearrange("p (hh c) -> p hh c", hh=8)
                a4 = acc[:].rearrange("p (hh c) -> p hh c", hh=8)
                nc.vector.tensor_copy(t4[:, :, 0:64], a4[:])
                nc.vector.tensor_copy(t4[:, :, 64:128], of4[:])
                vt[jb] = t_

            # ---- prologue: chunk 0 QKV production
            xs_cur = xs0
            qt_cur = [emit_qk_group(xs_cur, m, 0) for m in range(4)]
            for m in range(4, 8):
                emit_qk_group(xs_cur, m, 0)
            for ts in range(4):
                emit_v_group(xs_cur, 0, ts)

            for tc_i in range(NTC):
                if tc_i + 1 < NTC:
                    xs_next = emit_x_loads(tc_i + 1)
                    qt_next = [None] * 4
                else:
                    xs_next = None
                    qt_next = None

                # ---- attention for i-chunk ci = tc_i
                ysb_list = [None] * 4
                njb = 4 * tc_i + 4

                def emit_s(m, qtm, jb):
                    st = ps.tile([128, 1024], F32, tag="st", name="st")
                    for h in range(2):
                        nc.tensor.matmul(
                            st[:, h * 512 : h * 512 + 512],
                            kt[m][
                                h * 64 : h * 64 + 64,
                                jb * 128 : (jb + 1) * 128,
                            ],
                            qtm[h * 64 : h * 64 + 64, :],
                            start=True,
                            stop=True,
                        )
                    return st

                for m in range(4):
                    qtm = qt_cur[m]
                    ya = pyd.tile([128, 512], F32, tag="ya")
                    yb = pyd.tile([128, 512], F32, tag="yb")
                    st_next = emit_s(m, qtm, 0)
                    for jb in range(njb):
                        st = st_next
                        pt = ppt.tile([128, 1024], F16, tag="pt", name="pt")
                        if jb >= 4 * tc_i:
                            # causal mask: the q<j cutoff lies inside one
                            # 128-col window per head.  Memset the all-masked
                            # prefix (runs during the exp), exp only the
                            # suffix, affine_select only the window.
                            r = jb - 4 * tc_i
                            pt4 = pt[:].rearrange("p (h c) -> p h c", h=2)
                            st4 = st[:].rearrange("p (h c) -> p h c", h=2)
                            if r:
                                nc.gpsimd.memset(pt4[:, :, 0 : 128 * r], 0.0)
                                nc.scalar.activation(
                                    pt4[:, :, 128 * r : 512],
                                    st4[:, :, 128 * r : 512],
                                    EXP,
                                    scale=SCALE,
                                )
                            else:
                                nc.scalar.activation(
                                    pt[:], st[:], EXP, scale=SCALE
                                )
                            if jb + 1 < njb:
                                st_next = emit_s(m, qtm, jb + 1)
                            win = pt4[:, :, 128 * r : 128 * r + 128]
                            nc.gpsimd.affine_select(
                                out=win,
                                in_=win,
                                compare_op=mybir.AluOpType.is_ge,
                                fill=0.0,
                                base=0,
                                pattern=[[0, 2], [1, 128]],
                                channel_multiplier=-1,
                            )
                        else:
                            nc.scalar.activation(pt[:], st[:], EXP, scale=SCALE)
                            if jb + 1 < njb:
                                st_next = emit_s(m, qtm, jb + 1)
                        first, last = (jb == 0), (jb == njb - 1)
                        nc.tensor.matmul(
                            ya[:],
                            vt[jb][:, m * 256 : m * 256 + 128],
                            pt[:, 0:512],
                            start=first,
                            stop=last,
                        )
                        nc.tensor.matmul(
                            yb[:],
                            vt[jb][:, m * 256 + 128 : m * 256 + 256],
                            pt[:, 512:1024],
                            start=first,
                            stop=last,
                        )

                    # next chunk's QKV production, interleaved per pair:
                    # the in-order PE queue chews these while the attention
                    # phase is ACT-bound and while pair m's normalize chain
                    # drains (qt_cur[m] was just released by its last S)
                    if xs_next is not None:
                        qt_next[m] = emit_qk_group(xs_next, m, tc_i + 1)
                        emit_qk_group(xs_next, m + 4, tc_i + 1)
                        emit_v_group(xs_next, tc_i + 1, m)

                    # normalize: 1/den rows live at PSUM row 64 of ya/yb
                    # (row copies on ScE so the vector FIFO stays clear for
                    # the filler's qt/kt/vt copies)
                    rca = pr.tile([128, 512], F32, tag="rca", bufs=1)
                    nc.scalar.copy(rca[64:65, :], ya[64:65, :])
                    rcb = pr.tile([128, 512], F32, tag="rcb", bufs=1)
                    nc.scalar.copy(rcb[64:65, :], yb[64:65, :])
                    rc0 = pr.tile([1, 1024], F32, tag="rc0", bufs=1)
                    nc.sync.dma_start(rc0[0:1, 0:512], rca[64:65, :])
                    nc.sync.dma_start(rc0[0:1, 512:1024], rcb[64:65, :])
                    nc.vector.reciprocal_approx_fast(
                        rca[0:1, :], rc0[0:1, 0:512]
                    )
                    nc.vector.reciprocal_approx_fast(
                        rcb[0:1, :], rc0[0:1, 512:1024]
                    )
                    rba = pr.tile([128, 512], F32, tag="rba", bufs=2)
                    nc.gpsimd.partition_broadcast(rba[0:64, :], rca[0:1, :])
                    rbb = pr.tile([128, 512], F32, tag="rbb", bufs=2)
                    nc.gpsimd.partition_broadcast(rbb[0:64, :], rcb[0:1, :])
                    ytmp = pr.tile([128, 512], F16, tag="ytmp", bufs=1)
                    nc.vector.tensor_mul(
                        ytmp[0:64, :], yb[0:64, :], rbb[0:64, :]
                    )
                    ysb = pysb.tile([128, 512], F16, tag="ysb", name="ysb")
                    nc.vector.tensor_mul(
                        ysb[0:64, :], ya[0:64, :], rba[0:64, :]
                    )
                    nc.sync.dma_start(ysb[64:128, :], ytmp[0:64, :])
                    ysb_list[m] = ysb

                # ---- out-projection for i-chunk tc_i
                for ib in range(4):
                    for fh in range(2):
                        zp = ps.tile([128, 1024], F32, tag="st", name="zp")
                        zp = zp[:, 0:512]
                        for m in range(4):
                            nc.tensor.matmul(
                                zp[:],
                                ysb_list[m][:, ib * 128 : (ib + 1) * 128],
                                wo[m][:, fh * 512 : fh * 512 + 512],
                                start=(m == 0),
                                stop=(m == 3),
                            )
                        zsb = pzsb.tile([128, 512], F16, tag="zsb", bufs=2)
                        nc.vector.tensor_copy(zsb[:], zp[:])
                        row = (4 * tc_i + ib) * 128
                        nc.sync.dma_start(
                            z[row : row + 128, fh * 512 : fh * 512 + 512],
                            zsb[:],
                        )

                qt_cur = qt_next
                xs_cur = xs_next

    nc.finalize()
    _NC_CACHE["nc"] = nc
    return nc


def _in_maps(x, Wqkv, Wout):
    x = np.asarray(x, dtype=np.float32)
    Wqkv = np.asarray(Wqkv, dtype=np.float32)
    Wout = np.asarray(Wout, dtype=np.float32)
    xTs = [np.ascontiguousarray(x[b].T.astype(np.float16)) for b in range(B)]
    maps = []
    for c in range(8):
        b, g = divmod(c, 2)
        qrows = Wqkv[E * g : E * g + E]
        krows = Wqkv[D + E * g : D + E * g + E]
        vrows = Wqkv[2 * D + E * g : 2 * D + E * g + E]
        maps.append(
            {
                "xT": xTs[b],
                "wqkT": np.ascontiguousarray(
                    np.concatenate([qrows, krows], axis=0).T.astype(np.float16)
                ),
                "wvT": np.ascontiguousarray(vrows.T.astype(np.float16)),
                "woT": np.ascontiguousarray(
                    Wout[:, E * g : E * g + E].T.astype(np.float16)
                ),
            }
        )
    return maps


def _run(x, Wqkv, Wout, trace=False):
    from concourse.bass_utils import run_bass_kernel_spmd

    nc = build()
    res = run_bass_kernel_spmd(
        nc, _in_maps(x, Wqkv, Wout), core_ids=list(range(8)), trace=trace
    )
    out = np.empty((B, T, D), dtype=np.float32)
    for b in range(B):
        out[b] = res.results[2 * b]["z"].astype(np.float32) + res.results[
            2 * b + 1
        ]["z"].astype(np.float32)
    return out, res


def kernel(x, Wqkv, Wout):
    out, _ = _run(x, Wqkv, Wout, trace=False)
    return out


# revision 16
# speedup vs baseline: 1.2943x; 1.0738x over previous
"""Multi-head causal attention on 8 TRN2 NeuronCores.

Problem: x[4,2048,1024] @ Wqkv.T -> 16-head causal attention -> @ Wout.T.

Sharding: core c handles batch b=c//2, head-group g=c%2 (8 heads of 64).
Each core computes qkv for its (batch, head-group) slice, causal attention,
and a partial out-projection over its 512 columns of Wout's input dim.
Host sums the two partials per batch (the all-reduce of the hint).

Per-core layouts (host pre-transposes so every matmul contraction dim lands
on SBUF partitions):
  xT   [1024 d, 2048 t]      wqkT [1024 d, 1024 (q|k)e]
  wvT  [1024 d,  512 e]      woT  [ 512 e, 1024 f]
All tensors are fp16 (PSUM accumulation stays fp32): same 1-row/cycle PE
rate as fp32r but FWL halves LDWEIGHTS, DMA bytes halve, and the PE power
draw drops below the SW-throttle threshold that cost fp32r ~75us of K=4/8
clock-gating.  Simulated end-to-end fp16 error: 5.7e-4 rel (gate: 2e-2).

Emission is software-pipelined to keep the PE dense: the QKV-production
matmul groups for t-chunk tc+1 are interleaved into the attention phase of
chunk tc as PE filler; S^T for jb+1 is emitted before AV of jb so the PE
never waits on the ACT exp.  S head-pairs run concurrently on row-groups
0:63 / 64:127 (auto tile_position from the 64-partition APs).
"""

import sys

sys.path.insert(0, "/opt/trn_rl_repo")

import numpy as np

B, T, D, H = 4, 2048, 1024, 16
E = 512  # per-core head width (8 heads x 64)
ND = 8  # d chunks of 128
NTC = 4  # t chunks of 512
SCALE = 0.125  # 1/sqrt(64)

_NC_CACHE = {}


def build():
    if "nc" in _NC_CACHE:
        return _NC_CACHE["nc"]
    import concourse.bacc as bacc
    import concourse.mybir as mybir
    import concourse.tile as tile

    F32 = mybir.dt.float32
    F16 = mybir.dt.float16
    EXP = mybir.ActivationFunctionType.Exp

    nc = bacc.Bacc("TRN2", target_bir_lowering=False, debug=False, num_devices=8)
    xT = nc.declare_dram_parameter("xT", [D, T], F16, isOutput=False)
    wqkT = nc.declare_dram_parameter("wqkT", [D, 2 * E], F16, isOutput=False)
    wvT = nc.declare_dram_parameter("wvT", [D, E], F16, isOutput=False)
    woT = nc.declare_dram_parameter("woT", [E, D], F16, isOutput=False)
    z = nc.declare_dram_parameter("z", [T, D], F16, isOutput=True)

    with tile.TileContext(nc) as tc:
        with (
            tc.tile_pool(name="pw", bufs=8) as pw,
            tc.tile_pool(name="pwo", bufs=4) as pwo,
            tc.tile_pool(name="px", bufs=16) as px,
            tc.tile_pool(name="pkt", bufs=4) as pkt,
            tc.tile_pool(name="pqt", bufs=8) as pqt,
            tc.tile_pool(name="pv", bufs=16) as pv,
            tc.tile_pool(name="ppt", bufs=2) as ppt,
            tc.tile_pool(name="pr", bufs=2) as pr,
            tc.tile_pool(name="pysb", bufs=4) as pysb,
            tc.tile_pool(name="pzsb", bufs=1) as pzsb,
            tc.tile_pool(name="pone", bufs=1) as pone,
            tc.tile_pool(name="ps", bufs=2, space="PSUM") as ps,
            tc.tile_pool(name="pyd", bufs=2, space="PSUM") as pyd,
        ):
            # ---- first x chunk + qk weights interleaved so the first QKV
            # matmul is gated on ~1MB of DMA, not the whole 4.5MB weight set
            wqk = []
            xs0 = []
            for dc in range(ND):
                x_ = px.tile([128, 512], F16, tag="x", name="xs")
                nc.sync.dma_start(x_[:], xT[dc * 128 : (dc + 1) * 128, 0:512])
                xs0.append(x_)
                t_ = pw.tile([128, 2 * E], F16, tag="wqk")
                nc.sync.dma_start(t_[:], wqkT[dc * 128 : (dc + 1) * 128, :])
                wqk.append(t_)
            wv = []
            for dc in range(ND):
                t_ = pw.tile([128, E], F16, tag="wv")
                nc.sync.dma_start(t_[:], wvT[dc * 128 : (dc + 1) * 128, :])
                wv.append(t_)
            wo = []
            for m in range(4):
                t_ = pwo.tile([128, D], F16, tag="wo")
                nc.sync.dma_start(t_[:], woT[m * 128 : (m + 1) * 128, :])
                wo.append(t_)

            # per-head filler block for the AV stationary: [ones(32)|zeros(32)]
            ones_f = pone.tile([128, 512], F16, tag="onef")
            of4 = ones_f[:].rearrange("p (hh c) -> p hh c", hh=8)
            nc.gpsimd.memset(of4[:, :, 0:32], 1.0)
            nc.gpsimd.memset(of4[:, :, 32:64], 0.0)

            # persistent K^T [e,t] tiles; pair m = heads 2m / 2m+1 at
            # partition rows 0:64 / 64:128
            kt = [
                pkt.tile([128, T], F16, tag="kt", name=f"kt{i}")
                for i in range(4)
            ]
            vt = [None] * 16  # V tiles per 128-row t-block

            def emit_x_loads(tci):
                xs = []
                t0 = tci * 512
                for dc in range(ND):
                    t_ = px.tile([128, 512], F16, tag="x", name="xs")
                    nc.sync.dma_start(
                        t_[:], xT[dc * 128 : (dc + 1) * 128, t0 : t0 + 512]
                    )
                    xs.append(t_)
                return xs

            def emit_qk_group(xs, m, tci):
                """m 0..3: Q chunk -> returns qt tile; 4..7: K chunk."""
                acc = ps.tile([128, 1024], F32, tag="st", name="acc")
                acc = acc[:, 0:512]
                for dc in range(ND):
                    nc.tensor.matmul(
                        acc[:],
                        wqk[dc][:, m * 128 : (m + 1) * 128],
                        xs[dc][:],
                        start=(dc == 0),
                        stop=(dc == ND - 1),
                    )
                if m < 4:
                    t_ = pqt.tile([128, 512], F16, tag="qt", name="qt")
                    nc.vector.tensor_copy(t_[:], acc[:])
                    return t_
                t0 = tci * 512
                nc.vector.tensor_copy(kt[m - 4][:, t0 : t0 + 512], acc[:])
                return None

            def emit_v_group(xs, tci, ts):
                """V tile layout per head: [V_h(64) | ones(32) | zeros(32)]
                -> AV lhsT slices are 128 cols (FWL) and put Y at PSUM rows
                0:64, the denominator at 64:96, zeros at 96:128."""
                jb = 4 * tci + ts
                acc = ps.tile([128, 1024], F32, tag="st", name="vacc")
                acc = acc[:, 0:512]
                for dc in range(ND):
                    nc.tensor.matmul(
                        acc[:],
                        xs[dc][:, ts * 128 : (ts + 1) * 128],
                        wv[dc][:],
                        start=(dc == 0),
                        stop=(dc == ND - 1),
                    )
                t_ = pv.tile([128, 1024], F16, tag="v", name="vt")
                t4 = t_[:].rearrange("p (hh c) -> p hh c", hh=8)
                a4 = acc[:].rearrange("p (hh c) -> p hh c", hh=8)
                nc.vector.tensor_copy(t4[:, :, 0:64], a4[:])
                nc.vector.tensor_copy(t4[:, :, 64:128], of4[:])
                vt[jb] = t_

            def emit_outproj_block(ysbs, tci, ib, fh):
                zp = ps.tile([128, 1024], F32, tag="st", name="zp")
                zp = zp[:, 0:512]
                for m in range(4):
                    nc.tensor.matmul(
                        zp[:],
                        ysbs[m][:, ib * 128 : (ib + 1) * 128],
                        wo[m][:, fh * 512 : fh * 512 + 512],
                        start=(m == 0),
                        stop=(m == 3),
                    )
                zsb = pzsb.tile([128, 512], F16, tag="zsb", bufs=2)
                nc.vector.tensor_copy(zsb[:], zp[:])
                row = (4 * tci + ib) * 128
                nc.sync.dma_start(
                    z[row : row + 128, fh * 512 : fh * 512 + 512], zsb[:]
                )

            # ---- prologue: minimum to start pair 0 of chunk 0 (q0, k0,
            # all four V blocks); q1-q3/k1-k3 are produced as in-pair filler
            xs_cur = xs0
            qt_cur = [None] * 4
            qt_cur[0] = emit_qk_group(xs_cur, 0, 0)
            emit_qk_group(xs_cur, 4, 0)
            for ts in range(4):
                emit_v_group(xs_cur, 0, ts)

            # out-projection blocks of chunk tc-1, emitted one per jb
            # iteration inside chunk tc's first attention pair
            pending = []

            for tc_i in range(NTC):
                if tc_i + 1 < NTC:
                    xs_next = emit_x_loads(tc_i + 1)
                    qt_next = [None] * 4
                else:
                    xs_next = None
                    qt_next = None

                # ---- attention for i-chunk ci = tc_i
                ysb_list = [None] * 4
                njb = 4 * tc_i + 4

                def emit_s(m, qtm, jb):
                    st = ps.tile([128, 1024], F32, tag="st", name="st")
                    for h in range(2):
                        nc.tensor.matmul(
                            st[:, h * 512 : h * 512 + 512],
                            kt[m][
                                h * 64 : h * 64 + 64,
                                jb * 128 : (jb + 1) * 128,
                            ],
                            qtm[h * 64 : h * 64 + 64, :],
                            start=True,
                            stop=True,
                        )
                    return st

                for m in range(4):
                    qtm = qt_cur[m]
                    ya = pyd.tile([128, 512], F32, tag="ya")
                    yb = pyd.tile([128, 512], F32, tag="yb")
                    st_next = emit_s(m, qtm, 0)
                    for jb in range(njb):
                        st = st_next
                        pt = ppt.tile([128, 1024], F16, tag="pt", name="pt")
                        if jb >= 4 * tc_i:
                            # causal mask: the q<j cutoff lies inside one
                            # 128-col window per head.  Memset the all-masked
                            # prefix (runs during the exp), exp only the
                            # suffix, affine_select only the window.
                            r = jb - 4 * tc_i
                            pt4 = pt[:].rearrange("p (h c) -> p h c", h=2)
                            st4 = st[:].rearrange("p (h c) -> p h c", h=2)
                            if r:
                                nc.gpsimd.memset(pt4[:, :, 0 : 128 * r], 0.0)
                                nc.scalar.activation(
                                    pt4[:, :, 128 * r : 512],
                                    st4[:, :, 128 * r : 512],
                                    EXP,
                                    scale=SCALE,
                                )
                            else:
                                nc.scalar.activation(
                                    pt[:], st[:], EXP, scale=SCALE
                                )
                            if jb + 1 < njb:
                                st_next = emit_s(m, qtm, jb + 1)
                            win = pt4[:, :, 128 * r : 128 * r + 128]
                            nc.gpsimd.affine_select(
                                out=win,
                                in_=win,
                                compare_op=mybir.AluOpType.is_ge,
                                fill=0.0,
                                base=0,
                                pattern=[[0, 2], [1, 128]],
                                channel_multiplier=-1,
                            )
                        else:
                            nc.scalar.activation(pt[:], st[:], EXP, scale=SCALE)
                            if jb + 1 < njb:
                                st_next = emit_s(m, qtm, jb + 1)
                        first, last = (jb == 0), (jb == njb - 1)
                        nc.tensor.matmul(
                            ya[:],
                            vt[jb][:, m * 256 : m * 256 + 128],
                            pt[:, 0:512],
                            start=first,
                            stop=last,
                        )
                        nc.tensor.matmul(
                            yb[:],
                            vt[jb][:, m * 256 + 128 : m * 256 + 256],
                            pt[:, 512:1024],
                            start=first,
                            stop=last,
                        )
                        if pending:
                            args = pending.pop(0)
                            emit_outproj_block(*args)

                    # chunk 0: produce the next pair's q/k first (needed
                    # one pair later)
                    if tc_i == 0 and m < 3:
                        qt_cur[m + 1] = emit_qk_group(xs_cur, m + 1, 0)
                        emit_qk_group(xs_cur, m + 5, 0)

                    # next chunk's QKV production, interleaved per pair:
                    # the in-order PE queue chews these while the attention
                    # phase is ACT-bound and while pair m's normalize chain
                    # drains (qt_cur[m] was just released by its last S)
                    if xs_next is not None:
                        qt_next[m] = emit_qk_group(xs_next, m, tc_i + 1)
                        emit_qk_group(xs_next, m + 4, tc_i + 1)
                        emit_v_group(xs_next, tc_i + 1, m)

                    # normalize: 1/den rows live at PSUM row 64 of ya/yb
                    # (row copies on ScE so the vector FIFO stays clear for
                    # the filler's qt/kt/vt copies)
                    rca = pr.tile([128, 512], F32, tag="rca", bufs=1)
                    nc.scalar.copy(rca[64:65, :], ya[64:65, :])
                    rcb = pr.tile([128, 512], F32, tag="rcb", bufs=1)
                    nc.scalar.copy(rcb[64:65, :], yb[64:65, :])
                    rc0 = pr.tile([1, 1024], F32, tag="rc0", bufs=1)
                    nc.sync.dma_start(rc0[0:1, 0:512], rca[64:65, :])
                    nc.sync.dma_start(rc0[0:1, 512:1024], rcb[64:65, :])
                    nc.vector.reciprocal_approx_fast(
                        rca[0:1, :], rc0[0:1, 0:512]
                    )
                    nc.vector.reciprocal_approx_fast(
                        rcb[0:1, :], rc0[0:1, 512:1024]
                    )
                    rba = pr.tile([128, 512], F32, tag="rba", bufs=2)
                    nc.gpsimd.partition_broadcast(rba[0:64, :], rca[0:1, :])
                    rbb = pr.tile([128, 512], F32, tag="rbb", bufs=2)
                    nc.gpsimd.partition_broadcast(rbb[0:64, :], rcb[0:1, :])
                    ytmp = pr.tile([128, 512], F16, tag="ytmp", bufs=1)
                    nc.vector.tensor_mul(
                        ytmp[0:64, :], yb[0:64, :], rbb[0:64, :]
                    )
                    ysb = pysb.tile([128, 512], F16, tag="ysb", name="ysb")
                    nc.vector.tensor_mul(
                        ysb[0:64, :], ya[0:64, :], rba[0:64, :]
                    )
                    nc.sync.dma_start(ysb[64:128, :], ytmp[0:64, :])
                    ysb_list[m] = ysb

                # ---- out-projection for i-chunk tc_i: deferred into the
                # next chunk's first attention pair (last chunk: emit now)
                blocks = [
                    (ysb_list, tc_i, ib, fh)
                    for ib in range(4)
                    for fh in range(2)
                ]
                if tc_i + 1 < NTC:
                    pending = blocks
                else:
                    for args in blocks:
                        emit_outproj_block(*args)

                qt_cur = qt_next
                xs_cur = xs_next

    nc.finalize()
    _NC_CACHE["nc"] = nc
    return nc


def _in_maps(x, Wqkv, Wout):
    x = np.asarray(x, dtype=np.float32)
    Wqkv = np.asarray(Wqkv, dtype=np.float32)
    Wout = np.asarray(Wout, dtype=np.float32)
    xTs = [np.ascontiguousarray(x[b].T.astype(np.float16)) for b in range(B)]
    maps = []
    for c in range(8):
        b, g = divmod(c, 2)
        qrows = Wqkv[E * g : E * g + E]
        krows = Wqkv[D + E * g : D + E * g + E]
        vrows = Wqkv[2 * D + E * g : 2 * D + E * g + E]
        maps.append(
            {
                "xT": xTs[b],
                "wqkT": np.ascontiguousarray(
                    np.concatenate([qrows, krows], axis=0).T.astype(np.float16)
                ),
                "wvT": np.ascontiguousarray(vrows.T.astype(np.float16)),
                "woT": np.ascontiguousarray(
                    Wout[:, E * g : E * g + E].T.astype(np.float16)
                ),
            }
        )
    return maps


def _run(x, Wqkv, Wout, trace=False):
    from concourse.bass_utils import run_bass_kernel_spmd

    nc = build()
    res = run_bass_kernel_spmd(
        nc, _in_maps(x, Wqkv, Wout), core_ids=list(range(8)), trace=trace
    )
    out = np.empty((B, T, D), dtype=np.float32)
    for b in range(B):
        out[b] = res.results[2 * b]["z"].astype(np.float32) + res.results[
            2 * b + 1
        ]["z"].astype(np.float32)
    return out, res


def kernel(x, Wqkv, Wout):
    out, _ = _run(x, Wqkv, Wout, trace=False)
    return out


# revision 17
# speedup vs baseline: 1.3065x; 1.0095x over previous
"""Multi-head causal attention on 8 TRN2 NeuronCores.

Problem: x[4,2048,1024] @ Wqkv.T -> 16-head causal attention -> @ Wout.T.

Sharding: core c handles batch b=c//2, head-group g=c%2 (8 heads of 64).
Each core computes qkv for its (batch, head-group) slice, causal attention,
and a partial out-projection over its 512 columns of Wout's input dim.
Host sums the two partials per batch (the all-reduce of the hint).

Per-core layouts (host pre-transposes so every matmul contraction dim lands
on SBUF partitions):
  xT   [1024 d, 2048 t]      wqkT [1024 d, 1024 (q|k)e]
  wvT  [1024 d,  512 e]      woT  [ 512 e, 1024 f]
All tensors are fp16 (PSUM accumulation stays fp32): same 1-row/cycle PE
rate as fp32r but FWL halves LDWEIGHTS, DMA bytes halve, and the PE power
draw stays under the SW-throttle threshold that cost fp32r ~75us of K=4/8
clock-gating.  Simulated end-to-end fp16 error: 5.7e-4 rel (gate: 2e-2).

Schedule: a single flat software pipeline over (pair, j-block) iterations.
S for iteration i+1 is emitted before AV of iteration i (crossing pair
boundaries, so ACT never waits on a pair refill); next-chunk QKV production
groups and the previous chunk's out-projection blocks are sprinkled one
per iteration as PE filler; weights/x arrive via 5 consolidated strided
DMAs (sync-queue trigger rate, not bandwidth, gated the old prologue).
S head-pairs run concurrently on row-groups 0:63/64:127 (auto
tile_position from the 64-partition APs).
"""

import sys

sys.path.insert(0, "/opt/trn_rl_repo")

import numpy as np

B, T, D, H = 4, 2048, 1024, 16
E = 512  # per-core head width (8 heads x 64)
ND = 8  # d chunks of 128
NTC = 4  # t chunks of 512
SCALE = 0.125  # 1/sqrt(64)

_NC_CACHE = {}


def build():
    if "nc" in _NC_CACHE:
        return _NC_CACHE["nc"]
    import concourse.bacc as bacc
    import concourse.mybir as mybir
    import concourse.tile as tile

    F32 = mybir.dt.float32
    F16 = mybir.dt.float16
    EXP = mybir.ActivationFunctionType.Exp

    nc = bacc.Bacc("TRN2", target_bir_lowering=False, debug=False, num_devices=8)
    xT = nc.declare_dram_parameter("xT", [D, T], F16, isOutput=False)
    wqkT = nc.declare_dram_parameter("wqkT", [D, 2 * E], F16, isOutput=False)
    wvT = nc.declare_dram_parameter("wvT", [D, E], F16, isOutput=False)
    woT = nc.declare_dram_parameter("woT", [E, D], F16, isOutput=False)
    z = nc.declare_dram_parameter("z", [T, D], F16, isOutput=True)

    with tile.TileContext(nc) as tc:
        with (
            tc.tile_pool(name="pw", bufs=1) as pw,
            tc.tile_pool(name="px", bufs=2) as px,
            tc.tile_pool(name="pkt", bufs=4) as pkt,
            tc.tile_pool(name="pqt", bufs=8) as pqt,
            tc.tile_pool(name="pv", bufs=16) as pv,
            tc.tile_pool(name="ppt", bufs=2) as ppt,
            tc.tile_pool(name="pr", bufs=2) as pr,
            tc.tile_pool(name="pysb", bufs=4) as pysb,
            tc.tile_pool(name="pzsb", bufs=1) as pzsb,
            tc.tile_pool(name="pone", bufs=1) as pone,
            tc.tile_pool(name="ps", bufs=2, space="PSUM") as ps,
            tc.tile_pool(name="pyd", bufs=2, space="PSUM") as pyd,
        ):
            # ---- consolidated input DMAs (one trigger each; the sync
            # queue issues triggers at only ~0.65us apiece)
            wqk = pw.tile([128, ND * 2 * E], F16, tag="wqk")
            wqk3 = wqk[:].rearrange("p (dc e) -> p dc e", dc=ND)
            xs0 = px.tile([128, ND * 512], F16, tag="x", name="xs")
            wv = pw.tile([128, ND * E], F16, tag="wv")
            wo = pw.tile([128, 4 * D], F16, tag="wo")

            nc.sync.dma_start(
                wqk3[:, 0:4, :],
                wqkT[0:512, :].rearrange("(dc p) e -> p dc e", p=128),
            )
            nc.sync.dma_start(
                xs0[:].rearrange("p (dc t) -> p dc t", dc=ND),
                xT[:, 0:512].rearrange("(dc p) t -> p dc t", p=128),
            )
            nc.sync.dma_start(
                wqk3[:, 4:8, :],
                wqkT[512:1024, :].rearrange("(dc p) e -> p dc e", p=128),
            )
            nc.sync.dma_start(
                wv[:].rearrange("p (dc e) -> p dc e", dc=ND),
                wvT[:].rearrange("(dc p) e -> p dc e", p=128),
            )
            nc.sync.dma_start(
                wo[:].rearrange("p (m f) -> p m f", m=4),
                woT[:].rearrange("(m p) f -> p m f", p=128),
            )

            # per-head filler block for the AV stationary: [ones(32)|zeros(32)]
            ones_f = pone.tile([128, 512], F16, tag="onef")
            of4 = ones_f[:].rearrange("p (hh c) -> p hh c", hh=8)
            nc.gpsimd.memset(of4[:, :, 0:32], 1.0)
            nc.gpsimd.memset(of4[:, :, 32:64], 0.0)

            # persistent K^T [e,t] tiles; pair m = heads 2m / 2m+1 at
            # partition rows 0:64 / 64:128
            kt = [
                pkt.tile([128, T], F16, tag="kt", name=f"kt{i}")
                for i in range(4)
            ]
            vt = [None] * 16  # V tiles per 128-row t-block

            def emit_x_load(tci):
                t_ = px.tile([128, ND * 512], F16, tag="x", name="xs")
                t0 = tci * 512
                nc.sync.dma_start(
                    t_[:].rearrange("p (dc t) -> p dc t", dc=ND),
                    xT[:, t0 : t0 + 512].rearrange(
                        "(dc p) t -> p dc t", p=128
                    ),
                )
                return t_

            def emit_qk_group(xs, m, tci):
                """m 0..3: Q chunk -> returns qt tile; 4..7: K chunk."""
                acc = ps.tile([128, 1024], F32, tag="st", name="acc")
                acc = acc[:, 0:512]
                for dc in range(ND):
                    nc.tensor.matmul(
                        acc[:],
                        wqk[:, dc * 1024 + m * 128 : dc * 1024 + (m + 1) * 128],
                        xs[:, dc * 512 : (dc + 1) * 512],
                        start=(dc == 0),
                        stop=(dc == ND - 1),
                    )
                if m < 4:
                    t_ = pqt.tile([128, 512], F16, tag="qt", name="qt")
                    nc.vector.tensor_copy(t_[:], acc[:])
                    return t_
                t0 = tci * 512
                nc.vector.tensor_copy(kt[m - 4][:, t0 : t0 + 512], acc[:])
                return None

            def emit_v_group(xs, tci, ts):
                """V tile layout per head: [V_h(64) | ones(32) | zeros(32)]
                -> AV lhsT slices are 128 cols (FWL) and put Y at PSUM rows
                0:64, the denominator at 64:96, zeros at 96:128."""
                jb = 4 * tci + ts
                acc = ps.tile([128, 1024], F32, tag="st", name="vacc")
                acc = acc[:, 0:512]
                for dc in range(ND):
                    nc.tensor.matmul(
                        acc[:],
                        xs[:, dc * 512 + ts * 128 : dc * 512 + (ts + 1) * 128],
                        wv[:, dc * 512 : (dc + 1) * 512],
                        start=(dc == 0),
                        stop=(dc == ND - 1),
                    )
                t_ = pv.tile([128, 1024], F16, tag="v", name="vt")
                t4 = t_[:].rearrange("p (hh c) -> p hh c", hh=8)
                a4 = acc[:].rearrange("p (hh c) -> p hh c", hh=8)
                nc.vector.tensor_copy(t4[:, :, 0:64], a4[:])
                nc.vector.tensor_copy(t4[:, :, 64:128], of4[:])
                vt[jb] = t_

            def emit_outproj_block(ysbs, tci, ib, fh):
                zp = ps.tile([128, 1024], F32, tag="st", name="zp")
                zp = zp[:, 0:512]
                for m in range(4):
                    nc.tensor.matmul(
                        zp[:],
                        ysbs[m][:, ib * 128 : (ib + 1) * 128],
                        wo[:, m * 1024 + fh * 512 : m * 1024 + fh * 512 + 512],
                        start=(m == 0),
                        stop=(m == 3),
                    )
                zsb = pzsb.tile([128, 512], F16, tag="zsb", bufs=2)
                nc.vector.tensor_copy(zsb[:], zp[:])
                row = (4 * tci + ib) * 128
                nc.sync.dma_start(
                    z[row : row + 128, fh * 512 : fh * 512 + 512], zsb[:]
                )

            def emit_normalize(m, ya, yb):
                """ysb[0:64]=ya[0:64]/den_a, ysb[64:128]=yb[0:64]/den_b;
                dens live at PSUM row 64.  Row copies on ScE so the vector
                FIFO stays clear for the filler's qt/kt/vt copies."""
                rca = pr.tile([128, 512], F32, tag="rca", bufs=1)
                nc.scalar.copy(rca[64:65, :], ya[64:65, :])
                rcb = pr.tile([128, 512], F32, tag="rcb", bufs=1)
                nc.scalar.copy(rcb[64:65, :], yb[64:65, :])
                rc0 = pr.tile([1, 1024], F32, tag="rc0", bufs=1)
                nc.sync.dma_start(rc0[0:1, 0:512], rca[64:65, :])
                nc.sync.dma_start(rc0[0:1, 512:1024], rcb[64:65, :])
                nc.vector.reciprocal_approx_fast(rca[0:1, :], rc0[0:1, 0:512])
                nc.vector.reciprocal_approx_fast(
                    rcb[0:1, :], rc0[0:1, 512:1024]
                )
                rba = pr.tile([128, 512], F32, tag="rba", bufs=2)
                nc.gpsimd.partition_broadcast(rba[0:64, :], rca[0:1, :])
                rbb = pr.tile([128, 512], F32, tag="rbb", bufs=2)
                nc.gpsimd.partition_broadcast(rbb[0:64, :], rcb[0:1, :])
                ytmp = pr.tile([128, 512], F16, tag="ytmp", bufs=1)
                nc.vector.tensor_mul(ytmp[0:64, :], yb[0:64, :], rbb[0:64, :])
                ysb = pysb.tile([128, 512], F16, tag="ysb", name="ysb")
                nc.vector.tensor_mul(ysb[0:64, :], ya[0:64, :], rba[0:64, :])
                nc.sync.dma_start(ysb[64:128, :], ytmp[0:64, :])
                return ysb

            # ---- prologue: minimum to start pair 0 of chunk 0 (q0, k0,
            # all four V blocks); q1-q3/k1-k3 are produced as in-pair filler
            xs_cur = xs0
            qt_cur = [None] * 4
            qt_cur[0] = emit_qk_group(xs_cur, 0, 0)
            emit_qk_group(xs_cur, 4, 0)
            for ts in range(4):
                emit_v_group(xs_cur, 0, ts)

            # out-projection blocks of chunk tc-1, drained one per
            # iteration inside chunk tc's attention pipeline
            pending = []

            for tc_i in range(NTC):
                if tc_i + 1 < NTC:
                    xs_next = emit_x_load(tc_i + 1)
                    qt_next = [None] * 4
                else:
                    xs_next = None
                    qt_next = None

                # ---- attention for i-chunk ci = tc_i: flat pipeline
                ysb_list = [None] * 4
                njb = 4 * tc_i + 4
                seq = [(m, jb) for m in range(4) for jb in range(njb)]

                def emit_s(m, jb):
                    qtm = qt_cur[m]
                    st = ps.tile([128, 1024], F32, tag="st", name="st")
                    for h in range(2):
                        nc.tensor.matmul(
                            st[:, h * 512 : h * 512 + 512],
                            kt[m][
                                h * 64 : h * 64 + 64,
                                jb * 128 : (jb + 1) * 128,
                            ],
                            qtm[h * 64 : h * 64 + 64, :],
                            start=True,
                            stop=True,
                        )
                    return st

                ya = yb = None
                yab = [None] * 4
                st_next = emit_s(0, 0)
                for idx, (m, jb) in enumerate(seq):
                    if jb == 0:
                        ya = pyd.tile([128, 512], F32, tag="ya")
                        yb = pyd.tile([128, 512], F32, tag="yb")
                        yab[m] = (ya, yb)
                    st = st_next
                    pt = ppt.tile([128, 1024], F16, tag="pt", name="pt")
                    if jb >= 4 * tc_i:
                        # causal mask: the q<j cutoff lies inside one
                        # 128-col window per head.  Memset the all-masked
                        # prefix (runs during the exp), exp only the
                        # suffix, affine_select only the window.
                        r = jb - 4 * tc_i
                        pt4 = pt[:].rearrange("p (h c) -> p h c", h=2)
                        st4 = st[:].rearrange("p (h c) -> p h c", h=2)
                        if r:
                            nc.gpsimd.memset(pt4[:, :, 0 : 128 * r], 0.0)
                            nc.scalar.activation(
                                pt4[:, :, 128 * r : 512],
                                st4[:, :, 128 * r : 512],
                                EXP,
                                scale=SCALE,
                            )
                        else:
                            nc.scalar.activation(pt[:], st[:], EXP, scale=SCALE)
                        masked = True
                    else:
                        nc.scalar.activation(pt[:], st[:], EXP, scale=SCALE)
                        masked = False
                    if idx + 1 < len(seq):
                        st_next = emit_s(*seq[idx + 1])
                    if masked:
                        win = pt4[:, :, 128 * r : 128 * r + 128]
                        nc.gpsimd.affine_select(
                            out=win,
                            in_=win,
                            compare_op=mybir.AluOpType.is_ge,
                            fill=0.0,
                            base=0,
                            pattern=[[0, 2], [1, 128]],
                            channel_multiplier=-1,
                        )
                    first, last = (jb == 0), (jb == njb - 1)
                    nc.tensor.matmul(
                        ya[:],
                        vt[jb][:, m * 256 : m * 256 + 128],
                        pt[:, 0:512],
                        start=first,
                        stop=last,
                    )
                    nc.tensor.matmul(
                        yb[:],
                        vt[jb][:, m * 256 + 128 : m * 256 + 256],
                        pt[:, 512:1024],
                        start=first,
                        stop=last,
                    )

                    # ---- PE filler, one group per iteration
                    if pending:
                        emit_outproj_block(*pending.pop(0))
                    if tc_i == 0 and m < 3:
                        # chunk 0 bootstraps its own q/k pipeline
                        if jb == 0:
                            qt_cur[m + 1] = emit_qk_group(xs_cur, m + 1, 0)
                        elif jb == 1:
                            emit_qk_group(xs_cur, m + 5, 0)
                    if xs_next is not None:
                        if jb == njb - 3:
                            qt_next[m] = emit_qk_group(xs_next, m, tc_i + 1)
                        elif jb == njb - 2:
                            emit_qk_group(xs_next, m + 4, tc_i + 1)
                        elif jb == njb - 1:
                            emit_v_group(xs_next, tc_i + 1, m)
                    if last:
                        ysb_list[m] = emit_normalize(m, *yab[m])

                # ---- out-projection for i-chunk tc_i: deferred into the
                # next chunk's attention pipeline (last chunk: emit now)
                blocks = [
                    (ysb_list, tc_i, ib, fh)
                    for ib in range(4)
                    for fh in range(2)
                ]
                if tc_i + 1 < NTC:
                    pending = blocks
                else:
                    for args in blocks:
                        emit_outproj_block(*args)

                qt_cur = qt_next
                xs_cur = xs_next

    nc.finalize()
    _NC_CACHE["nc"] = nc
    return nc


def _in_maps(x, Wqkv, Wout):
    x = np.asarray(x, dtype=np.float32)
    Wqkv = np.asarray(Wqkv, dtype=np.float32)
    Wout = np.asarray(Wout, dtype=np.float32)
    xTs = [np.ascontiguousarray(x[b].T.astype(np.float16)) for b in range(B)]
    maps = []
    for c in range(8):
        b, g = divmod(c, 2)
        qrows = Wqkv[E * g : E * g + E]
        krows = Wqkv[D + E * g : D + E * g + E]
        vrows = Wqkv[2 * D + E * g : 2 * D + E * g + E]
        maps.append(
            {
                "xT": xTs[b],
                "wqkT": np.ascontiguousarray(
                    np.concatenate([qrows, krows], axis=0).T.astype(np.float16)
                ),
                "wvT": np.ascontiguousarray(vrows.T.astype(np.float16)),
                "woT": np.ascontiguousarray(
                    Wout[:, E * g : E * g + E].T.astype(np.float16)
                ),
            }
        )
    return maps


def _run(x, Wqkv, Wout, trace=False):
    from concourse.bass_utils import run_bass_kernel_spmd

    nc = build()
    res = run_bass_kernel_spmd(
        nc, _in_maps(x, Wqkv, Wout), core_ids=list(range(8)), trace=trace
    )
    out = np.empty((B, T, D), dtype=np.float32)
    for b in range(B):
        out[b] = res.results[2 * b]["z"].astype(np.float32) + res.results[
            2 * b + 1
        ]["z"].astype(np.float32)
    return out, res


def kernel(x, Wqkv, Wout):
    out, _ = _run(x, Wqkv, Wout, trace=False)
    return out


# revision 22
# speedup vs baseline: 1.3612x; 1.0419x over previous
"""Multi-head causal attention on 8 TRN2 NeuronCores.

Problem: x[4,2048,1024] @ Wqkv.T -> 16-head causal attention -> @ Wout.T.

Sharding: core c handles batch b=c//2, head-group g=c%2 (8 heads of 64).
Each core computes qkv for its (batch, head-group) slice, causal attention,
and a partial out-projection over its 512 columns of Wout's input dim.
Host sums the two partials per batch (the all-reduce of the hint).

Per-core layouts (host pre-transposes so every matmul contraction dim lands
on SBUF partitions):
  xT   [1024 d, 2048 t]      wqkT [1024 d, 1024 (q|k)e]
  wvT  [1024 d,  512 e]      woT  [ 512 e, 1024 f]
All tensors are fp16 (PSUM accumulation stays fp32): same 1-row/cycle PE
rate as fp32r but FWL halves LDWEIGHTS, DMA bytes halve, and the PE power
draw stays under the SW-throttle threshold that cost fp32r ~75us of K=4/8
clock-gating.  Simulated end-to-end fp16 error: 5.7e-4 rel (gate: 2e-2).

Schedule: a single flat software pipeline over (pair, j-block) iterations.
S for iteration i+1 is emitted before AV of iteration i (crossing pair
boundaries, so ACT never waits on a pair refill); next-chunk QKV production
groups and the previous chunk's out-projection blocks are sprinkled one
per iteration as PE filler; weights/x arrive via 5 consolidated strided
DMAs (sync-queue trigger rate, not bandwidth, gated the old prologue).
S head-pairs run concurrently on row-groups 0:63/64:127 (auto
tile_position from the 64-partition APs).
"""

import sys

sys.path.insert(0, "/opt/trn_rl_repo")

import numpy as np

B, T, D, H = 4, 2048, 1024, 16
E = 512  # per-core head width (8 heads x 64)
ND = 8  # d chunks of 128
NTC = 4  # t chunks of 512
SCALE = 0.125  # 1/sqrt(64)

_NC_CACHE = {}


def build():
    if "nc" in _NC_CACHE:
        return _NC_CACHE["nc"]
    import concourse.bacc as bacc
    import concourse.mybir as mybir
    import concourse.tile as tile

    F32 = mybir.dt.float32
    F16 = mybir.dt.float16
    EXP = mybir.ActivationFunctionType.Exp

    nc = bacc.Bacc("TRN2", target_bir_lowering=False, debug=False, num_devices=8)
    xT = nc.declare_dram_parameter("xT", [D, T], F16, isOutput=False)
    wqkT = nc.declare_dram_parameter("wqkT", [D, 2 * E], F16, isOutput=False)
    wvT = nc.declare_dram_parameter("wvT", [D, E], F16, isOutput=False)
    woT = nc.declare_dram_parameter("woT", [E, D], F16, isOutput=False)
    z = nc.declare_dram_parameter("z", [T, D], F16, isOutput=True)

    with tile.TileContext(nc) as tc:
        with (
            tc.tile_pool(name="pw", bufs=1) as pw,
            tc.tile_pool(name="px", bufs=2) as px,
            tc.tile_pool(name="pkt", bufs=4) as pkt,
            tc.tile_pool(name="pqt", bufs=8) as pqt,
            tc.tile_pool(name="pv", bufs=16) as pv,
            tc.tile_pool(name="ppt", bufs=2) as ppt,
            tc.tile_pool(name="pr", bufs=2) as pr,
            tc.tile_pool(name="pysb", bufs=8) as pysb,
            tc.tile_pool(name="pzsb", bufs=1) as pzsb,
            tc.tile_pool(name="pone", bufs=1) as pone,
            tc.tile_pool(name="ps", bufs=2, space="PSUM") as ps,
            tc.tile_pool(name="pyd", bufs=2, space="PSUM") as pyd,
        ):
            # ---- consolidated input DMAs (one trigger each; the sync
            # queue issues triggers at only ~0.65us apiece)
            wqk = pw.tile([128, ND * 2 * E], F16, tag="wqk")
            wqk3 = wqk[:].rearrange("p (dc e) -> p dc e", dc=ND)
            xs0 = px.tile([128, ND * 512], F16, tag="x", name="xs")
            wv = pw.tile([128, ND * E], F16, tag="wv")
            wo = pw.tile([128, 4 * D], F16, tag="wo")

            nc.sync.dma_start(
                wqk3[:, 0:4, :],
                wqkT[0:512, :].rearrange("(dc p) e -> p dc e", p=128),
            )
            nc.sync.dma_start(
                xs0[:].rearrange("p (dc t) -> p dc t", dc=ND),
                xT[:, 0:512].rearrange("(dc p) t -> p dc t", p=128),
            )
            nc.sync.dma_start(
                wqk3[:, 4:8, :],
                wqkT[512:1024, :].rearrange("(dc p) e -> p dc e", p=128),
            )
            nc.sync.dma_start(
                wv[:].rearrange("p (dc e) -> p dc e", dc=ND),
                wvT[:].rearrange("(dc p) e -> p dc e", p=128),
            )
            nc.sync.dma_start(
                wo[:].rearrange("p (m f) -> p m f", m=4),
                woT[:].rearrange("(m p) f -> p m f", p=128),
            )

            # per-head filler block for the AV stationary: [ones(32)|zeros(32)]
            ones_f = pone.tile([128, 512], F16, tag="onef")
            of4 = ones_f[:].rearrange("p (hh c) -> p hh c", hh=8)
            nc.gpsimd.memset(of4[:, :, 0:32], 1.0)
            nc.gpsimd.memset(of4[:, :, 32:64], 0.0)

            # persistent K^T [e,t] tiles; pair m = heads 2m / 2m+1 at
            # partition rows 0:64 / 64:128
            kt = [
                pkt.tile([128, T], F16, tag="kt", name=f"kt{i}")
                for i in range(4)
            ]
            vt = [None] * 16  # V tiles per 128-row t-block

            def emit_x_load(tci):
                t_ = px.tile([128, ND * 512], F16, tag="x", name="xs")
                t0 = tci * 512
                nc.sync.dma_start(
                    t_[:].rearrange("p (dc t) -> p dc t", dc=ND),
                    xT[:, t0 : t0 + 512].rearrange(
                        "(dc p) t -> p dc t", p=128
                    ),
                )
                return t_

            def emit_qk_group(xs, m, tci):
                """m 0..3: Q chunk -> returns qt tile; 4..7: K chunk."""
                acc = ps.tile([128, 1024], F32, tag="st", name="acc")
                acc = acc[:, 0:512]
                for dc in range(ND):
                    nc.tensor.matmul(
                        acc[:],
                        wqk[:, dc * 1024 + m * 128 : dc * 1024 + (m + 1) * 128],
                        xs[:, dc * 512 : (dc + 1) * 512],
                        start=(dc == 0),
                        stop=(dc == ND - 1),
                    )
                if m < 4:
                    t_ = pqt.tile([128, 512], F16, tag="qt", name="qt")
                    nc.vector.tensor_copy(t_[:], acc[:])
                    return t_
                t0 = tci * 512
                nc.vector.tensor_copy(kt[m - 4][:, t0 : t0 + 512], acc[:])
                return None

            def emit_v_group(xs, tci, ts):
                """V tile layout per head: [V_h(64) | ones(32) | zeros(32)]
                -> AV lhsT slices are 128 cols (FWL) and put Y at PSUM rows
                0:64, the denominator at 64:96, zeros at 96:128."""
                jb = 4 * tci + ts
                acc = ps.tile([128, 1024], F32, tag="st", name="vacc")
                acc = acc[:, 0:512]
                for dc in range(ND):
                    nc.tensor.matmul(
                        acc[:],
                        xs[:, dc * 512 + ts * 128 : dc * 512 + (ts + 1) * 128],
                        wv[:, dc * 512 : (dc + 1) * 512],
                        start=(dc == 0),
                        stop=(dc == ND - 1),
                    )
                t_ = pv.tile([128, 1024], F16, tag="v", name="vt")
                t4 = t_[:].rearrange("p (hh c) -> p hh c", hh=8)
                a4 = acc[:].rearrange("p (hh c) -> p hh c", hh=8)
                nc.vector.tensor_copy(t4[:, :, 0:64], a4[:])
                nc.vector.tensor_copy(t4[:, :, 64:128], of4[:])
                vt[jb] = t_

            def emit_outproj_block(ysbs, tci, ib, fh):
                zp = ps.tile([128, 1024], F32, tag="st", name="zp")
                zp = zp[:, 0:512]
                for m in range(4):
                    nc.tensor.matmul(
                        zp[:],
                        ysbs[m][:, ib * 128 : (ib + 1) * 128],
                        wo[:, m * 1024 + fh * 512 : m * 1024 + fh * 512 + 512],
                        start=(m == 0),
                        stop=(m == 3),
                    )
                zsb = pzsb.tile([128, 512], F16, tag="zsb", bufs=2)
                nc.vector.tensor_copy(zsb[:], zp[:])
                row = (4 * tci + ib) * 128
                nc.sync.dma_start(
                    z[row : row + 128, fh * 512 : fh * 512 + 512], zsb[:]
                )

            def emit_normalize(m, ya, yb):
                """ysb[0:64]=ya[0:64]/den_a, ysb[64:128]=yb[0:64]/den_b;
                dens live at PSUM row 64.  The reciprocal reads PSUM
                directly (keeps both the scalar and vector FIFOs clear of
                row-copy ops that would stall the next pair's ACT)."""
                rca = pr.tile([128, 512], F32, tag="rca", bufs=1)
                nc.vector.tensor_copy(rca[64:65, :], ya[64:65, :])
                rcb = pr.tile([128, 512], F32, tag="rcb", bufs=1)
                nc.vector.tensor_copy(rcb[64:65, :], yb[64:65, :])
                rc0 = pr.tile([1, 1024], F32, tag="rc0", bufs=1)
                nc.sync.dma_start(rc0[0:1, 0:512], rca[64:65, :])
                nc.sync.dma_start(rc0[0:1, 512:1024], rcb[64:65, :])
                nc.vector.reciprocal_approx_fast(rca[0:1, :], rc0[0:1, 0:512])
                nc.vector.reciprocal_approx_fast(
                    rcb[0:1, :], rc0[0:1, 512:1024]
                )
                rba = pr.tile([128, 512], F32, tag="rba", bufs=2)
                nc.gpsimd.partition_broadcast(rba[0:64, :], rca[0:1, :])
                rbb = pr.tile([128, 512], F32, tag="rbb", bufs=2)
                nc.gpsimd.partition_broadcast(rbb[0:64, :], rcb[0:1, :])
                ytmp = pr.tile([128, 512], F16, tag="ytmp", bufs=1)
                nc.vector.tensor_mul(ytmp[0:64, :], yb[0:64, :], rbb[0:64, :])
                ysb = pysb.tile([128, 512], F16, tag="ysb", name="ysb")
                nc.vector.tensor_mul(ysb[0:64, :], ya[0:64, :], rba[0:64, :])
                nc.sync.dma_start(ysb[64:128, :], ytmp[0:64, :])
                return ysb

            # ---- prologue: minimum to start pair 0 of chunk 0 (q0, k0,
            # all four V blocks); q1-q3/k1-k3 are produced as in-pair filler
            xs_cur = xs0
            qt_cur = [None] * 4
            qt_cur[0] = emit_qk_group(xs_cur, 0, 0)
            emit_qk_group(xs_cur, 4, 0)
            for ts in range(4):
                emit_v_group(xs_cur, 0, ts)

            # out-projection blocks of chunk tc-1, drained one per
            # iteration inside chunk tc's attention pipeline
            pending = []

            for tc_i in range(NTC):
                if tc_i + 1 < NTC:
                    xs_next = emit_x_load(tc_i + 1)
                    qt_next = [None] * 4
                else:
                    xs_next = None
                    qt_next = None

                # ---- attention for i-chunk ci = tc_i: flat pipeline
                ysb_list = [None] * 4
                njb = 4 * tc_i + 4
                seq = [(m, jb) for m in range(4) for jb in range(njb)]

                def emit_s(m, jb):
                    qtm = qt_cur[m]
                    st = ps.tile([128, 1024], F32, tag="st", name="st")
                    for h in range(2):
                        nc.tensor.matmul(
                            st[:, h * 512 : h * 512 + 512],
                            kt[m][
                                h * 64 : h * 64 + 64,
                                jb * 128 : (jb + 1) * 128,
                            ],
                            qtm[h * 64 : h * 64 + 64, :],
                            start=True,
                            stop=True,
                        )
                    return st

                ya = yb = None
                yab = [None] * 4
                st_next = emit_s(0, 0)
                for idx, (m, jb) in enumerate(seq):
                    if jb == 0:
                        ya = pyd.tile([128, 512], F32, tag="ya")
                        yb = pyd.tile([128, 512], F32, tag="yb")
                        yab[m] = (ya, yb)
                    st = st_next
                    pt = ppt.tile([128, 1024], F16, tag="pt", name="pt")
                    if jb >= 4 * tc_i:
                        # causal mask: the q<j cutoff lies inside one
                        # 128-col window per head.  Memset the all-masked
                        # prefix (runs during the exp), exp only the
                        # suffix, affine_select only the window.
                        r = jb - 4 * tc_i
                        pt4 = pt[:].rearrange("p (h c) -> p h c", h=2)
                        st4 = st[:].rearrange("p (h c) -> p h c", h=2)
                        if r:
                            nc.gpsimd.memset(pt4[:, :, 0 : 128 * r], 0.0)
                            nc.scalar.activation(
                                pt4[:, :, 128 * r : 512],
                                st4[:, :, 128 * r : 512],
                                EXP,
                                scale=SCALE,
                            )
                        else:
                            nc.scalar.activation(pt[:], st[:], EXP, scale=SCALE)
                        masked = True
                    else:
                        nc.scalar.activation(pt[:], st[:], EXP, scale=SCALE)
                        masked = False
                    if idx + 1 < len(seq):
                        st_next = emit_s(*seq[idx + 1])
                    if masked:
                        win = pt4[:, :, 128 * r : 128 * r + 128]
                        nc.gpsimd.affine_select(
                            out=win,
                            in_=win,
                            compare_op=mybir.AluOpType.is_ge,
                            fill=0.0,
                            base=0,
                            pattern=[[0, 2], [1, 128]],
                            channel_multiplier=-1,
                        )
                    first, last = (jb == 0), (jb == njb - 1)
                    nc.tensor.matmul(
                        ya[:],
                        vt[jb][:, m * 256 : m * 256 + 128],
                        pt[:, 0:512],
                        start=first,
                        stop=last,
                    )
                    nc.tensor.matmul(
                        yb[:],
                        vt[jb][:, m * 256 + 128 : m * 256 + 256],
                        pt[:, 512:1024],
                        start=first,
                        stop=last,
                    )

                    # ---- PE filler, one group per iteration (outproj
                    # blocks wait a few iterations so the previous chunk's
                    # last ysb DMA lands before a block can head-of-line
                    # block the tensor queue)
                    if pending and idx >= 5:
                        emit_outproj_block(*pending.pop(0))
                    if tc_i == 0 and m < 3:
                        # chunk 0 bootstraps its own q/k pipeline
                        if jb == 0:
                            qt_cur[m + 1] = emit_qk_group(xs_cur, m + 1, 0)
                        elif jb == 1:
                            emit_qk_group(xs_cur, m + 5, 0)
                    if xs_next is not None:
                        if jb == njb - 3:
                            qt_next[m] = emit_qk_group(xs_next, m, tc_i + 1)
                        elif jb == njb - 2:
                            emit_qk_group(xs_next, m + 4, tc_i + 1)
                        elif jb == njb - 1:
                            emit_v_group(xs_next, tc_i + 1, m)
                    if last:
                        ysb_list[m] = emit_normalize(m, *yab[m])

                # ---- out-projection for i-chunk tc_i: deferred into the
                # next chunk's attention pipeline (last chunk: emit now)
                blocks = [
                    (ysb_list, tc_i, ib, fh)
                    for ib in range(4)
                    for fh in range(2)
                ]
                if tc_i + 1 < NTC:
                    pending = blocks
                else:
                    for args in blocks:
                        emit_outproj_block(*args)

                qt_cur = qt_next
                xs_cur = xs_next

    nc.finalize()
    _NC_CACHE["nc"] = nc
    return nc


def _in_maps(x, Wqkv, Wout):
    x = np.asarray(x, dtype=np.float32)
    Wqkv = np.asarray(Wqkv, dtype=np.float32)
    Wout = np.asarray(Wout, dtype=np.float32)
    xTs = [np.ascontiguousarray(x[b].T.astype(np.float16)) for b in range(B)]
    maps = []
    for c in range(8):
        b, g = divmod(c, 2)
        qrows = Wqkv[E * g : E * g + E]
        krows = Wqkv[D + E * g : D + E * g + E]
        vrows = Wqkv[2 * D + E * g : 2 * D + E * g + E]
        maps.append(
            {
                "xT": xTs[b],
                "wqkT": np.ascontiguousarray(
                    np.concatenate([qrows, krows], axis=0).T.astype(np.float16)
                ),
                "wvT": np.ascontiguousarray(vrows.T.astype(np.float16)),
                "woT": np.ascontiguousarray(
                    Wout[:, E * g : E * g + E].T.astype(np.float16)
                ),
            }
        )
    return maps


def _run(x, Wqkv, Wout, trace=False):
    from concourse.bass_utils import run_bass_kernel_spmd

    nc = build()
    res = run_bass_kernel_spmd(
        nc, _in_maps(x, Wqkv, Wout), core_ids=list(range(8)), trace=trace
    )
    out = np.empty((B, T, D), dtype=np.float32)
    for b in range(B):
        out[b] = res.results[2 * b]["z"].astype(np.float32) + res.results[
            2 * b + 1
        ]["z"].astype(np.float32)
    return out, res


def kernel(x, Wqkv, Wout):
    out, _ = _run(x, Wqkv, Wout, trace=False)
    return out


# revision 27
# speedup vs baseline: 1.3997x; 1.0283x over previous
"""Multi-head causal attention on 8 TRN2 NeuronCores.

Problem: x[4,2048,1024] @ Wqkv.T -> 16-head causal attention -> @ Wout.T.

Sharding: core c handles batch b=c//2, head-group g=c%2 (8 heads of 64).
Each core computes qkv for its (batch, head-group) slice, causal attention,
and a partial out-projection over its 512 columns of Wout's input dim.
Host sums the two partials per batch (the all-reduce of the hint).

Per-core layouts (host pre-transposes so every matmul contraction dim lands
on SBUF partitions):
  xT   [1024 d, 2048 t]      wqkT [1024 d, 1024 (q|k)e]
  wvT  [1024 d,  512 e]      woT  [ 512 e, 1024 f]
All tensors are fp16 (PSUM accumulation stays fp32): same 1-row/cycle PE
rate as fp32r but FWL halves LDWEIGHTS, DMA bytes halve, and the PE power
draw stays under the SW-throttle threshold that cost fp32r ~75us of K=4/8
clock-gating.  Simulated end-to-end fp16 error: 5.7e-4 rel (gate: 2e-2).

Schedule: a single flat software pipeline over (pair, j-block) iterations.
S for iteration i+1 is emitted before AV of iteration i (crossing pair
boundaries, so ACT never waits on a pair refill); next-chunk QKV production
groups and the previous chunk's out-projection blocks are sprinkled one
per iteration as PE filler; weights/x arrive via 5 consolidated strided
DMAs (sync-queue trigger rate, not bandwidth, gated the old prologue).
S head-pairs run concurrently on row-groups 0:63/64:127 (auto
tile_position from the 64-partition APs).
"""

import sys

sys.path.insert(0, "/opt/trn_rl_repo")

import numpy as np

B, T, D, H = 4, 2048, 1024, 16
E = 512  # per-core head width (8 heads x 64)
ND = 8  # d chunks of 128
NTC = 4  # t chunks of 512
SCALE = 0.125  # 1/sqrt(64)

_NC_CACHE = {}


def build():
    if "nc" in _NC_CACHE:
        return _NC_CACHE["nc"]
    import concourse.bacc as bacc
    import concourse.mybir as mybir
    import concourse.tile as tile

    F32 = mybir.dt.float32
    F16 = mybir.dt.float16
    EXP = mybir.ActivationFunctionType.Exp

    nc = bacc.Bacc("TRN2", target_bir_lowering=False, debug=False, num_devices=8)
    xT = nc.declare_dram_parameter("xT", [D, T], F16, isOutput=False)
    wqkT = nc.declare_dram_parameter("wqkT", [D, 2 * E], F16, isOutput=False)
    wvT = nc.declare_dram_parameter("wvT", [D, E], F16, isOutput=False)
    woT = nc.declare_dram_parameter("woT", [E, D], F16, isOutput=False)
    z = nc.declare_dram_parameter("z", [T, D], F16, isOutput=True)

    with tile.TileContext(nc) as tc:
        with (
            tc.tile_pool(name="pw", bufs=1) as pw,
            tc.tile_pool(name="px", bufs=2) as px,
            tc.tile_pool(name="pkt", bufs=4) as pkt,
            tc.tile_pool(name="pqt", bufs=8) as pqt,
            tc.tile_pool(name="pv", bufs=16) as pv,
            tc.tile_pool(name="ppt", bufs=2) as ppt,
            tc.tile_pool(name="pr", bufs=2) as pr,
            tc.tile_pool(name="pysb", bufs=8) as pysb,
            tc.tile_pool(name="pzsb", bufs=1) as pzsb,
            tc.tile_pool(name="pone", bufs=1) as pone,
            tc.tile_pool(name="ps", bufs=2, space="PSUM") as ps,
            tc.tile_pool(name="pyd", bufs=2, space="PSUM") as pyd,
        ):
            # ---- consolidated input DMAs (one trigger each; the sync
            # queue issues triggers at only ~0.65us apiece)
            wqk = pw.tile([128, ND * 2 * E], F16, tag="wqk")
            wqk3 = wqk[:].rearrange("p (dc e) -> p dc e", dc=ND)
            xs0 = px.tile([128, ND * 512], F16, tag="x", name="xs")
            wv = pw.tile([128, ND * E], F16, tag="wv")
            wo = pw.tile([128, 4 * D], F16, tag="wo")

            nc.sync.dma_start(
                wqk3[:, 0:4, :],
                wqkT[0:512, :].rearrange("(dc p) e -> p dc e", p=128),
            )
            nc.sync.dma_start(
                xs0[:].rearrange("p (dc t) -> p dc t", dc=ND),
                xT[:, 0:512].rearrange("(dc p) t -> p dc t", p=128),
            )
            nc.sync.dma_start(
                wqk3[:, 4:8, :],
                wqkT[512:1024, :].rearrange("(dc p) e -> p dc e", p=128),
            )
            nc.sync.dma_start(
                wv[:].rearrange("p (dc e) -> p dc e", dc=ND),
                wvT[:].rearrange("(dc p) e -> p dc e", p=128),
            )
            nc.sync.dma_start(
                wo[:].rearrange("p (m f) -> p m f", m=4),
                woT[:].rearrange("(m p) f -> p m f", p=128),
            )

            # per-head filler block for the AV stationary: [ones(32)|zeros(32)]
            ones_f = pone.tile([128, 512], F16, tag="onef")
            of4 = ones_f[:].rearrange("p (hh c) -> p hh c", hh=8)
            nc.gpsimd.memset(of4[:, :, 0:32], 1.0)
            nc.gpsimd.memset(of4[:, :, 32:64], 0.0)

            # 0/1 upper-triangle (keep c>=j) mask for the causal window;
            # applied as a DVE multiply so gpsimd runs only its
            # partition_broadcast library (an affine_select/broadcast mix
            # thrashes the gpsimd custom-op library, ~6us per swap)
            mtri = pone.tile([128, 128], F16, tag="mtri")
            nc.vector.memset(mtri[:], 1.0)
            nc.gpsimd.affine_select(
                out=mtri[:],
                in_=mtri[:],
                compare_op=mybir.AluOpType.is_ge,
                fill=0.0,
                base=0,
                pattern=[[1, 128]],
                channel_multiplier=-1,
            )
            mtri3 = (
                mtri[:]
                .rearrange("p (o c) -> p o c", o=1)
                .broadcast_to((128, 2, 128))
            )

            # persistent K^T [e,t] tiles; pair m = heads 2m / 2m+1 at
            # partition rows 0:64 / 64:128
            kt = [
                pkt.tile([128, T], F16, tag="kt", name=f"kt{i}")
                for i in range(4)
            ]
            vt = [None] * 16  # V tiles per 128-row t-block

            def emit_x_load(tci):
                t_ = px.tile([128, ND * 512], F16, tag="x", name="xs")
                t0 = tci * 512
                nc.sync.dma_start(
                    t_[:].rearrange("p (dc t) -> p dc t", dc=ND),
                    xT[:, t0 : t0 + 512].rearrange(
                        "(dc p) t -> p dc t", p=128
                    ),
                )
                return t_

            def emit_qk_group(xs, m, tci):
                """m 0..3: Q chunk -> returns qt tile; 4..7: K chunk."""
                acc = ps.tile([128, 1024], F32, tag="st", name="acc")
                acc = acc[:, 0:512]
                for dc in range(ND):
                    nc.tensor.matmul(
                        acc[:],
                        wqk[:, dc * 1024 + m * 128 : dc * 1024 + (m + 1) * 128],
                        xs[:, dc * 512 : (dc + 1) * 512],
                        start=(dc == 0),
                        stop=(dc == ND - 1),
                    )
                if m < 4:
                    t_ = pqt.tile([128, 512], F16, tag="qt", name="qt")
                    nc.vector.tensor_copy(t_[:], acc[:])
                    return t_
                t0 = tci * 512
                nc.vector.tensor_copy(kt[m - 4][:, t0 : t0 + 512], acc[:])
                return None

            def emit_v_group(xs, tci, ts):
                """V tile layout per head: [V_h(64) | ones(32) | zeros(32)]
                -> AV lhsT slices are 128 cols (FWL) and put Y at PSUM rows
                0:64, the denominator at 64:96, zeros at 96:128."""
                jb = 4 * tci + ts
                acc = ps.tile([128, 1024], F32, tag="st", name="vacc")
                acc = acc[:, 0:512]
                for dc in range(ND):
                    nc.tensor.matmul(
                        acc[:],
                        xs[:, dc * 512 + ts * 128 : dc * 512 + (ts + 1) * 128],
                        wv[:, dc * 512 : (dc + 1) * 512],
                        start=(dc == 0),
                        stop=(dc == ND - 1),
                    )
                t_ = pv.tile([128, 1024], F16, tag="v", name="vt")
                t4 = t_[:].rearrange("p (hh c) -> p hh c", hh=8)
                a4 = acc[:].rearrange("p (hh c) -> p hh c", hh=8)
                nc.vector.tensor_copy(t4[:, :, 0:64], a4[:])
                nc.vector.tensor_copy(t4[:, :, 64:128], of4[:])
                vt[jb] = t_

            def emit_outproj_block(ysbs, tci, ib, fh):
                zp = ps.tile([128, 1024], F32, tag="st", name="zp")
                zp = zp[:, 0:512]
                for m in range(4):
                    nc.tensor.matmul(
                        zp[:],
                        ysbs[m][:, ib * 128 : (ib + 1) * 128],
                        wo[:, m * 1024 + fh * 512 : m * 1024 + fh * 512 + 512],
                        start=(m == 0),
                        stop=(m == 3),
                    )
                zsb = pzsb.tile([128, 512], F16, tag="zsb", bufs=2)
                nc.vector.tensor_copy(zsb[:], zp[:])
                row = (4 * tci + ib) * 128
                nc.sync.dma_start(
                    z[row : row + 128, fh * 512 : fh * 512 + 512], zsb[:]
                )

            def emit_normalize(m, ya, yb):
                """ysb[0:64]=ya[0:64]/den_a, ysb[64:128]=yb[0:64]/den_b;
                dens live at PSUM row 64.  The reciprocal reads PSUM
                directly (keeps both the scalar and vector FIFOs clear of
                row-copy ops that would stall the next pair's ACT)."""
                rca = pr.tile([128, 512], F32, tag="rca", bufs=1)
                nc.vector.tensor_copy(rca[64:65, :], ya[64:65, :])
                rcb = pr.tile([128, 512], F32, tag="rcb", bufs=1)
                nc.vector.tensor_copy(rcb[64:65, :], yb[64:65, :])
                rc0 = pr.tile([1, 1024], F32, tag="rc0", bufs=1)
                nc.sync.dma_start(rc0[0:1, 0:512], rca[64:65, :])
                nc.sync.dma_start(rc0[0:1, 512:1024], rcb[64:65, :])
                nc.vector.reciprocal_approx_fast(rca[0:1, :], rc0[0:1, 0:512])
                nc.vector.reciprocal_approx_fast(
                    rcb[0:1, :], rc0[0:1, 512:1024]
                )
                rba = pr.tile([128, 512], F32, tag="rba", bufs=2)
                nc.gpsimd.partition_broadcast(rba[0:64, :], rca[0:1, :])
                rbb = pr.tile([128, 512], F32, tag="rbb", bufs=2)
                nc.gpsimd.partition_broadcast(rbb[0:64, :], rcb[0:1, :])
                ytmp = pr.tile([128, 512], F16, tag="ytmp", bufs=1)
                nc.vector.tensor_mul(ytmp[0:64, :], yb[0:64, :], rbb[0:64, :])
                ysb = pysb.tile([128, 512], F16, tag="ysb", name="ysb")
                nc.vector.tensor_mul(ysb[0:64, :], ya[0:64, :], rba[0:64, :])
                nc.sync.dma_start(ysb[64:128, :], ytmp[0:64, :])
                return ysb

            # ---- prologue: minimum to start pair 0 of chunk 0 (q0, k0,
            # all four V blocks); q1-q3/k1-k3 are produced as in-pair filler
            xs_cur = xs0
            qt_cur = [None] * 4
            qt_cur[0] = emit_qk_group(xs_cur, 0, 0)
            emit_qk_group(xs_cur, 4, 0)
            for ts in range(4):
                emit_v_group(xs_cur, 0, ts)

            # out-projection blocks of chunk tc-1, drained one per
            # iteration inside chunk tc's attention pipeline
            pending = []

            for tc_i in range(NTC):
                if tc_i + 1 < NTC:
                    xs_next = emit_x_load(tc_i + 1)
                    qt_next = [None] * 4
                else:
                    xs_next = None
                    qt_next = None

                # ---- attention for i-chunk ci = tc_i: flat pipeline
                ysb_list = [None] * 4
                njb = 4 * tc_i + 4
                seq = [(m, jb) for m in range(4) for jb in range(njb)]

                def emit_s(m, jb):
                    qtm = qt_cur[m]
                    st = ps.tile([128, 1024], F32, tag="st", name="st")
                    for h in range(2):
                        nc.tensor.matmul(
                            st[:, h * 512 : h * 512 + 512],
                            kt[m][
                                h * 64 : h * 64 + 64,
                                jb * 128 : (jb + 1) * 128,
                            ],
                            qtm[h * 64 : h * 64 + 64, :],
                            start=True,
                            stop=True,
                        )
                    return st

                ya = yb = None
                yab = [None] * 4
                st_next = emit_s(0, 0)
                for idx, (m, jb) in enumerate(seq):
                    if jb == 0:
                        ya = pyd.tile([128, 512], F32, tag="ya")
                        yb = pyd.tile([128, 512], F32, tag="yb")
                        yab[m] = (ya, yb)
                    st = st_next
                    pt = ppt.tile([128, 1024], F16, tag="pt", name="pt")
                    if jb >= 4 * tc_i:
                        # causal mask: the q<j cutoff lies inside one
                        # 128-col window per head.  Memset the all-masked
                        # prefix (runs during the exp), exp only the
                        # suffix, affine_select only the window.
                        r = jb - 4 * tc_i
                        pt4 = pt[:].rearrange("p (h c) -> p h c", h=2)
                        st4 = st[:].rearrange("p (h c) -> p h c", h=2)
                        if r:
                            nc.vector.memset(pt4[:, :, 0 : 128 * r], 0.0)
                            nc.scalar.activation(
                                pt4[:, :, 128 * r : 512],
                                st4[:, :, 128 * r : 512],
                                EXP,
                                scale=SCALE,
                            )
                        else:
                            nc.scalar.activation(pt[:], st[:], EXP, scale=SCALE)
                        masked = True
                    else:
                        nc.scalar.activation(pt[:], st[:], EXP, scale=SCALE)
                        masked = False
                    if idx + 1 < len(seq):
                        st_next = emit_s(*seq[idx + 1])
                    if masked:
                        win = pt4[:, :, 128 * r : 128 * r + 128]
                        nc.vector.tensor_mul(win, win, mtri3)
                    first, last = (jb == 0), (jb == njb - 1)
                    nc.tensor.matmul(
                        ya[:],
                        vt[jb][:, m * 256 : m * 256 + 128],
                        pt[:, 0:512],
                        start=first,
                        stop=last,
                    )
                    nc.tensor.matmul(
                        yb[:],
                        vt[jb][:, m * 256 + 128 : m * 256 + 256],
                        pt[:, 512:1024],
                        start=first,
                        stop=last,
                    )

                    # ---- PE filler, one group per iteration (outproj
                    # blocks wait a few iterations so the previous chunk's
                    # last ysb DMA lands before a block can head-of-line
                    # block the tensor queue)
                    if pending and idx >= 5:
                        emit_outproj_block(*pending.pop(0))
                    if tc_i == 0 and m < 3:
                        # chunk 0 bootstraps its own q/k pipeline
                        if jb == 0:
                            qt_cur[m + 1] = emit_qk_group(xs_cur, m + 1, 0)
                        elif jb == 1:
                            emit_qk_group(xs_cur, m + 5, 0)
                    if xs_next is not None:
                        if jb == njb - 3:
                            qt_next[m] = emit_qk_group(xs_next, m, tc_i + 1)
                        elif jb == njb - 2:
                            emit_qk_group(xs_next, m + 4, tc_i + 1)
                        elif jb == njb - 1:
                            emit_v_group(xs_next, tc_i + 1, m)
                    if last:
                        ysb_list[m] = emit_normalize(m, *yab[m])

                # ---- out-projection for i-chunk tc_i: deferred into the
                # next chunk's attention pipeline (last chunk: emit now)
                blocks = [
                    (ysb_list, tc_i, ib, fh)
                    for ib in range(4)
                    for fh in range(2)
                ]
                if tc_i + 1 < NTC:
                    pending = blocks
                else:
                    for args in blocks:
                        emit_outproj_block(*args)

                qt_cur = qt_next
                xs_cur = xs_next

    nc.finalize()
    _NC_CACHE["nc"] = nc
    return nc


def _in_maps(x, Wqkv, Wout):
    x = np.asarray(x, dtype=np.float32)
    Wqkv = np.asarray(Wqkv, dtype=np.float32)
    Wout = np.asarray(Wout, dtype=np.float32)
    xTs = [np.ascontiguousarray(x[b].T.astype(np.float16)) for b in range(B)]
    maps = []
    for c in range(8):
        b, g = divmod(c, 2)
        qrows = Wqkv[E * g : E * g + E]
        krows = Wqkv[D + E * g : D + E * g + E]
        vrows = Wqkv[2 * D + E * g : 2 * D + E * g + E]
        maps.append(
            {
                "xT": xTs[b],
                "wqkT": np.ascontiguousarray(
                    np.concatenate([qrows, krows], axis=0).T.astype(np.float16)
                ),
                "wvT": np.ascontiguousarray(vrows.T.astype(np.float16)),
                "woT": np.ascontiguousarray(
                    Wout[:, E * g : E * g + E].T.astype(np.float16)
                ),
            }
        )
    return maps


def _run(x, Wqkv, Wout, trace=False):
    from concourse.bass_utils import run_bass_kernel_spmd

    nc = build()
    res = run_bass_kernel_spmd(
        nc, _in_maps(x, Wqkv, Wout), core_ids=list(range(8)), trace=trace
    )
    out = np.empty((B, T, D), dtype=np.float32)
    for b in range(B):
        out[b] = res.results[2 * b]["z"].astype(np.float32) + res.results[
            2 * b + 1
        ]["z"].astype(np.float32)
    return out, res


def kernel(x, Wqkv, Wout):
    out, _ = _run(x, Wqkv, Wout, trace=False)
    return out
